# revision 1
# baseline (speedup 1.0000x reference)
"""HardMaxAttention Trainium2 Bass kernel.

Reference computation (per batch b):
    Q = x @ W_Q.T            (T, 2)
    K = x @ W_K.T            (T, 2)
    scores = Q @ K.T         (T, T), causal-masked (strict upper tri = -inf)
    idx = argmax(scores, -1) (T,)
    out = x[idx] @ W_V.T     (T, D)   [== take_along_axis(V, idx)]

Sharding: 8 cores = 4 batches x 2 t-parity shards. Core c handles batch
b=c//2 and t-tiles of parity h=c%2 (t-tile th in {h, h+2, ..., h+30}).
Parity interleave balances the causal triangle across the two cores of a
batch.

One static SPMD program serves all cores: each core receives x[b] with
rows PERMUTED so its own 16 t-tiles occupy positions 0..2047 and the
other parity's tiles occupy 2048..4095.  For score tile i (true t-block
2i+h) the valid causal key range is true blocks 0..2i+h, which in
permuted coordinates is always positions [0,(i+1)*128) u
[2048, 2048+(i+1)*128) regardless of h.  The one h-dependent piece (is
the other-parity tile at position 2048+i*128 before or after the
diagonal?) is folded into an input mask tile.  Gather indices are
positions into the permuted x, so they never need to be mapped back.

The Q/K/scores path is strictly fp32: bf16 there flips ~90 argmaxes
(measured), which is catastrophic for a hard gather.  The V projection
may run in bf16 (USE_BF16_V) since it only affects output magnitude,
not the gather index.
"""

import numpy as np

B, T, D, H = 4, 4096, 1024, 2
P = 128
NT = T // P            # 32 t-tiles per batch
MYT = NT // 2          # 16 t-tiles per core
KD = D // P            # 8 contraction blocks
NG = T // 512          # 8 QK groups
N_CORES = 8
NEG = -1.0e30

USE_BF16_V = True      # bf16 gather + V projection (argmax path stays fp32)

_prog_cache = {}


def _build_program():
    """Build the single SPMD Bass program (same for every core)."""
    import concourse.bacc as bacc
    import concourse.mybir as mybir
    import concourse.tile as tile
    import concourse.bass as bass
    from concourse.masks import make_identity

    f32 = mybir.dt.float32
    u32 = mybir.dt.uint32
    vdt = mybir.dt.bfloat16 if USE_BF16_V else f32

    nc = bacc.Bacc(None, target_bir_lowering=False)

    # xq[g, p, k*512+c] = x_perm[g*512+c, k*128+p]: transposed layout in
    # groups of 512 rows -> QK runs as N=512 matmuls, no device transposes.
    xq = nc.dram_tensor("xq", [NG, P, KD * 512], f32, kind="ExternalInput")
    # gather + V-projection source (bf16 copy when USE_BF16_V)
    xv = nc.dram_tensor("xv", [T, D], vdt, kind="ExternalInput")
    w_qkT = nc.dram_tensor("w_qkT", [D, 4], f32, kind="ExternalInput")
    w_vT = nc.dram_tensor("w_vT", [D, D], vdt, kind="ExternalInput")
    dmask = nc.dram_tensor("dmask", [P, P], f32, kind="ExternalInput")
    tmask = nc.dram_tensor("tmask", [P, P], f32, kind="ExternalInput")
    out = nc.dram_tensor("out", [MYT, P, D], f32, kind="ExternalOutput")

    with tile.TileContext(nc) as tc:
        with (
            tc.tile_pool(name="const", bufs=1) as cpool,
            tc.tile_pool(name="xin", bufs=3) as xpool,
            tc.tile_pool(name="xt", bufs=2) as xtpool,
            tc.tile_pool(name="qk", bufs=1) as qkpool,
            tc.tile_pool(name="sc", bufs=2) as scpool,
            tc.tile_pool(name="small", bufs=4) as spool,
            tc.tile_pool(name="xg", bufs=3) as xgpool,
            tc.tile_pool(name="ob", bufs=3) as opool,
            tc.tile_pool(name="tp_ps", bufs=2, space="PSUM") as tpsum,
            tc.tile_pool(name="mm_ps", bufs=2, space="PSUM") as mmpsum,
            tc.tile_pool(name="vo_ps", bufs=2, space="PSUM") as vopsum,
        ):
            # ---- constants ----
            ident = cpool.tile([P, P], vdt)
            make_identity(nc, ident[:])
            wqk_sb = cpool.tile([P, 4 * KD], f32)
            for k in range(KD):
                nc.sync.dma_start(
                    wqk_sb[:, k * 4:(k + 1) * 4], w_qkT[k * P:(k + 1) * P, :]
                )
            dmask_sb = cpool.tile([P, P], f32)
            nc.sync.dma_start(dmask_sb[:], dmask[:])
            tmask_sb = cpool.tile([P, P], f32)
            nc.sync.dma_start(tmask_sb[:], tmask[:])

            # Q^T/K^T for all positions (separate tiles: matmul operands
            # must share base partition 0/32/64).
            qT = qkpool.tile([2, T], f32, tag="qT")
            kT = qkpool.tile([2, T], f32, tag="kT")

            # ---- phase 1: compute Q^T/K^T from pre-transposed xq ----
            # lhsT = W (2 cols -> cheap LDWEIGHTS), rhs = xq chunk (N=512).
            for g in range(NG):
                xq_sb = xpool.tile([P, KD * 512], f32)
                nc.sync.dma_start(xq_sb[:], xq[g, :, :])
                q_ps = mmpsum.tile([2, 512], f32, space="PSUM", tag="mmps")
                k_ps = mmpsum.tile([2, 512], f32, space="PSUM", tag="mmps")
                for k in range(KD):
                    nc.tensor.matmul(
                        q_ps[:],
                        lhsT=wqk_sb[:, k * 4:k * 4 + 2],
                        rhs=xq_sb[:, k * 512:(k + 1) * 512],
                        start=(k == 0),
                        stop=(k == KD - 1),
                    )
                for k in range(KD):
                    nc.tensor.matmul(
                        k_ps[:],
                        lhsT=wqk_sb[:, k * 4 + 2:k * 4 + 4],
                        rhs=xq_sb[:, k * 512:(k + 1) * 512],
                        start=(k == 0),
                        stop=(k == KD - 1),
                    )
                nc.vector.tensor_copy(qT[:, g * 512:(g + 1) * 512], q_ps[:])
                nc.vector.tensor_copy(kT[:, g * 512:(g + 1) * 512], k_ps[:])

            # W_V^T loaded late so phase-1 xq DMAs go first
            wv_sb = cpool.tile([P, KD * D], vdt)
            for k in range(KD):
                nc.sync.dma_start(
                    wv_sb[:, k * D:(k + 1) * D], w_vT[k * P:(k + 1) * P, :]
                )

            # ---- phase 2+3 per own t-tile ----
            for i in range(MYT):
                E = (i + 1) * P       # width of each of the two key ranges
                W = 2 * E
                sc = scpool.tile([P, 2 * MYT * P], f32)  # max width 4096

                # range A: own-parity keys, positions [0, E); diagonal
                # block is the last P columns -> add dmask there.
                # range B: other-parity keys, positions [2048, 2048+E),
                # written at columns [E, 2E); last P columns get tmask.
                nchunk = 0
                for (base_src, base_dst, mk) in (
                    (0, 0, dmask_sb),
                    (T // 2, E, tmask_sb),
                ):
                    for c0 in range(0, E, 512):
                        c1 = min(E, c0 + 512)
                        nn = c1 - c0
                        nchunk += 1
                        ps = mmpsum.tile([P, 512], f32, space="PSUM",
                                         tag="mmps")
                        nc.tensor.matmul(
                            ps[:, :nn],
                            lhsT=qT[:, i * P:(i + 1) * P],
                            rhs=kT[:, base_src + c0:base_src + c1],
                            start=True,
                            stop=True,
                        )
                        if c1 == E:
                            # chunk contains the masked block (last P cols)
                            if nn > P:
                                nc.vector.tensor_copy(
                                    sc[:, base_dst + c0:base_dst + c1 - P],
                                    ps[:, :nn - P],
                                )
                            nc.vector.tensor_tensor(
                                out=sc[:, base_dst + E - P:base_dst + E],
                                in0=ps[:, nn - P:nn],
                                in1=mk[:],
                                op=mybir.AluOpType.add,
                            )
                        else:
                            nc.vector.tensor_copy(
                                sc[:, base_dst + c0:base_dst + c1], ps[:, :nn]
                            )

                mx8 = spool.tile([P, 8], f32, tag="mx8")
                ix8 = spool.tile([P, 8], u32, tag="ix8")
                nc.vector.max(out=mx8[:], in_=sc[:, :W])
                nc.vector.max_index(out=ix8[:], in_max=mx8[:], in_values=sc[:, :W])

                # positions >= E belong to range B: add (2048 - E)
                idxf = spool.tile([P, 1], f32, tag="idxf")
                gef = spool.tile([P, 1], f32, tag="gef")
                idxu = spool.tile([P, 1], u32, tag="idxu")
                nc.vector.tensor_copy(idxf[:], ix8[:, 0:1])
                nc.vector.tensor_scalar(
                    gef[:], idxf[:], float(E), scalar2=None,
                    op0=mybir.AluOpType.is_ge,
                )
                nc.vector.tensor_scalar(
                    gef[:], gef[:], float(T // 2 - E), scalar2=None,
                    op0=mybir.AluOpType.mult,
                )
                nc.vector.tensor_tensor(
                    out=idxf[:], in0=idxf[:], in1=gef[:],
                    op=mybir.AluOpType.add,
                )
                nc.vector.tensor_copy(idxu[:], idxf[:])

                # gather the argmax rows of (permuted) x
                xg = xgpool.tile([P, D], vdt)
                nc.gpsimd.indirect_dma_start(
                    out=xg[:],
                    out_offset=None,
                    in_=xv[:],
                    in_offset=bass.IndirectOffsetOnAxis(ap=idxu[:, 0:1], axis=0),
                )

                # transpose gathered rows (PE transpose via identity)
                xgT = xtpool.tile([P, D], vdt, tag="xgt")
                for k in range(KD):
                    tp = tpsum.tile([P, P], vdt, space="PSUM", tag="tp")
                    nc.tensor.transpose(
                        tp[:], xg[:, k * P:(k + 1) * P], ident[:]
                    )
                    nc.vector.tensor_copy(xgT[:, k * P:(k + 1) * P], tp[:])

                # out tile = xg @ W_V.T  ->  (xgT).T @ w_vT
                vo = vopsum.tile([P, D], f32, space="PSUM")
                for n in range(2):
                    for k in range(KD):
                        nc.tensor.matmul(
                            vo[:, n * 512:(n + 1) * 512],
                            lhsT=xgT[:, k * P:(k + 1) * P],
                            rhs=wv_sb[:, k * D + n * 512:k * D + n * 512 + 512],
                            start=(k == 0),
                            stop=(k == KD - 1),
                        )
                ob = opool.tile([P, D], f32)
                nc.scalar.copy(ob[:], vo[:])
                nc.sync.dma_start(out[i, :, :], ob[:])

    nc.compile()
    return nc


def get_program():
    if "nc" not in _prog_cache:
        _prog_cache["nc"] = _build_program()
    return _prog_cache["nc"]


def make_core_inputs(x_full, W_Q, W_K, W_V):
    """Host-side shard: per-core input dicts (and the tile maps)."""
    import ml_dtypes
    vnp = ml_dtypes.bfloat16 if USE_BF16_V else np.float32

    x_full = np.ascontiguousarray(x_full, dtype=np.float32)
    w_qkT = np.ascontiguousarray(
        np.concatenate([W_Q, W_K], axis=0).T, dtype=np.float32
    )  # (D, 4)
    w_vT = np.ascontiguousarray(np.asarray(W_V, np.float32).T.astype(vnp))

    r = np.arange(P)
    dmask = np.where(r[None, :] <= r[:, None], 0.0, NEG).astype(np.float32)

    in_maps = []
    tiles_per_core = []
    for c in range(N_CORES):
        b, h = divmod(c, 2)
        mine = [2 * i + h for i in range(MYT)]
        other = [2 * i + (1 - h) for i in range(MYT)]
        rows = np.concatenate(
            [np.arange(t * P, (t + 1) * P) for t in mine + other]
        )
        xb_perm = np.ascontiguousarray(x_full[b][rows])
        # transposed group layout: xq[g, p, k*512+c] = xb_perm[g*512+c, k*128+p]
        xqg = np.ascontiguousarray(
            xb_perm.reshape(NG, 512, KD, P).transpose(0, 3, 2, 1)
            .reshape(NG, P, KD * 512)
        )
        # other-parity tile at position 2048+i*128 is true block 2i+(1-h):
        # h=0 -> block 2i+1 > diag 2i   -> fully masked
        # h=1 -> block 2i   < diag 2i+1 -> fully valid
        tmask = np.full((P, P), NEG if h == 0 else 0.0, dtype=np.float32)
        in_maps.append(
            {
                "xq": xqg,
                "xv": np.ascontiguousarray(xb_perm.astype(vnp)),
                "w_qkT": w_qkT,
                "w_vT": w_vT,
                "dmask": dmask,
                "tmask": tmask,
            }
        )
        tiles_per_core.append(mine)
    return in_maps, tiles_per_core


def assemble_output(results, tiles_per_core):
    out_full = np.empty((B, T, D), dtype=np.float32)
    for c in range(N_CORES):
        b = c // 2
        oc = results[c]["out"]
        for i, th in enumerate(tiles_per_core[c]):
            out_full[b, th * P:(th + 1) * P, :] = oc[i]
    return out_full


def kernel(**inputs):
    from concourse.bass_utils import run_bass_kernel_spmd

    x_full = np.asarray(inputs["x"], dtype=np.float32)
    in_maps, tiles_per_core = make_core_inputs(
        x_full, np.asarray(inputs["W_Q"]), np.asarray(inputs["W_K"]),
        np.asarray(inputs["W_V"])
    )
    nc = get_program()
    res = run_bass_kernel_spmd(nc, in_maps, core_ids=list(range(N_CORES)))
    return assemble_output(res.results, tiles_per_core)



# revision 18
# speedup vs baseline: 1.3424x; 1.3424x over previous
"""HardMaxAttention Trainium2 Bass kernel (v2: fp16 hi/lo QK + K=6 scores).

Reference computation (per batch b):
    Q = x @ W_Q.T            (T, 2)
    K = x @ W_K.T            (T, 2)
    scores = Q @ K.T         (T, T), causal-masked (strict upper tri = -inf)
    idx = argmax(scores, -1) (T,)
    out = x[idx] @ W_V.T     (T, D)   [== take_along_axis(V, idx)]

Sharding: 8 cores = 4 batches x 2 t-parity shards (as v1).  Core c gets
batch b=c//2, parity h=c%2; x[b] rows are permuted so own tiles occupy
positions 0..2047, other parity 2048..4095.

Precision scheme (the argmax is intolerant of low-precision scores --
bf16 flips ~90 rows, fp32r ~11; fp32 matmuls cost 4 cycles/row):
  - x and W_Q/W_K are split hi/lo into fp16 on the host (x = xh + xl
    exactly to ~2^-24).  Q^T/K^T accumulate in PSUM fp32 from 3 fp16
    matmul terms (Wh xh + Wh xl + Wl xh); error ~2^-24.
  - The PE computes q rows triplicated (M=6, lhsT cols [W,W,W]) so the
    stacked hi/lo score operands can be extracted with partition-aligned
    casts/subs only: qs = [qh qh ql] (rows 0:6), ks = [kh kl kh] (rows
    32:38 via tile_position col group 1).
  - scores = qh.kh + qh.kl + ql.kh as ONE K=6 fp16 matmul per chunk
    (1 cycle/row); error ~2^-22 -> no argmax flips.
  - V path in bf16; output stored bf16 and upcast on host.
"""

import numpy as np

B, T, D, H = 4, 4096, 1024, 2
P = 128
NT = T // P            # 32 t-tiles per batch
MYT = NT // 2          # 16 t-tiles per core
KD = D // P            # 8 contraction blocks
NG = T // 512          # 8 QK groups (4 own-parity, 4 other-parity)
N_CORES = 8
NEG = -1.0e30

_prog_cache = {}


def _build_program():
    import concourse.bacc as bacc
    import concourse.mybir as mybir
    import concourse.tile as tile
    import concourse.bass as bass
    from concourse.masks import make_identity

    f32 = mybir.dt.float32
    f16 = mybir.dt.float16
    bf16 = mybir.dt.bfloat16
    u32 = mybir.dt.uint32

    nc = bacc.Bacc(None, target_bir_lowering=False)

    # x^T in group layout, fp16 hi/lo: xq*[g, p, k*512+c] = x_perm[g*512+c, k*128+p]
    xqh = nc.dram_tensor("xqh", [NG, P, KD * 512], f16, kind="ExternalInput")
    xql = nc.dram_tensor("xql", [NG, P, KD * 512], f16, kind="ExternalInput")
    # gather + V-projection source
    xv = nc.dram_tensor("xv", [T, D], bf16, kind="ExternalInput")
    # weights [D, 6]: cols = [W W W] triplicated (2 cols each), hi/lo fp16
    wq6h = nc.dram_tensor("wq6h", [D, 6], f16, kind="ExternalInput")
    wq6l = nc.dram_tensor("wq6l", [D, 6], f16, kind="ExternalInput")
    wk6h = nc.dram_tensor("wk6h", [D, 6], f16, kind="ExternalInput")
    wk6l = nc.dram_tensor("wk6l", [D, 6], f16, kind="ExternalInput")
    w_vT = nc.dram_tensor("w_vT", [D, D], bf16, kind="ExternalInput")
    dmask = nc.dram_tensor("dmask", [P, P], f32, kind="ExternalInput")
    tmask = nc.dram_tensor("tmask", [P, P], f32, kind="ExternalInput")
    out = nc.dram_tensor("out", [MYT, P, D], bf16, kind="ExternalOutput")

    # group emission order: own-parity g alternating with other-parity g+4,
    # so tiles 4j..4j+3 unlock after pair (j, j+4).
    def gpair(j):
        return (j, j + 4)

    with tile.TileContext(nc) as tc:
        with (
            tc.tile_pool(name="const", bufs=1) as cpool,
            tc.tile_pool(name="xin", bufs=2) as xpool,
            tc.tile_pool(name="xt", bufs=2) as xtpool,
            tc.tile_pool(name="qk", bufs=1) as qkpool,
            tc.tile_pool(name="sc", bufs=2) as scpool,
            tc.tile_pool(name="small", bufs=4) as spool,
            tc.tile_pool(name="xg", bufs=3) as xgpool,
            tc.tile_pool(name="ob", bufs=3) as opool,
            tc.tile_pool(name="tp_ps", bufs=2, space="PSUM") as tpsum,
            tc.tile_pool(name="mm_ps", bufs=2, space="PSUM") as mmpsum,
            tc.tile_pool(name="vo_ps", bufs=2, space="PSUM") as vopsum,
        ):
            # ---- constants ----
            ident = cpool.tile([P, P], bf16)
            make_identity(nc, ident[:])
            wqh_sb = cpool.tile([P, 6 * KD], f16)
            wql_sb = cpool.tile([P, 6 * KD], f16)
            wkh_sb = cpool.tile([P, 6 * KD], f16)
            wkl_sb = cpool.tile([P, 6 * KD], f16)
            for k in range(KD):
                nc.sync.dma_start(wqh_sb[:, k * 6:(k + 1) * 6],
                                  wq6h[k * P:(k + 1) * P, :])
                nc.sync.dma_start(wql_sb[:, k * 6:(k + 1) * 6],
                                  wq6l[k * P:(k + 1) * P, :])
                nc.sync.dma_start(wkh_sb[:, k * 6:(k + 1) * 6],
                                  wk6h[k * P:(k + 1) * P, :])
                nc.sync.dma_start(wkl_sb[:, k * 6:(k + 1) * 6],
                                  wk6l[k * P:(k + 1) * P, :])
            dmask_sb = cpool.tile([P, P], f32)
            nc.sync.dma_start(dmask_sb[:], dmask[:])
            tmask_sb = cpool.tile([P, P], f32)
            nc.sync.dma_start(tmask_sb[:], tmask[:])

            # stacked hi/lo score operands (both base partition 0), paired
            # rows contract together: qs6 = [ql qh qh], ks6 = [kh kl kh]
            # -> ql.kh + qh.kl + qh.kh
            qs6 = qkpool.tile([6, T], f16, tag="qs6")
            ks6 = qkpool.tile([6, T], f16, tag="ks6")

            wv_sb = cpool.tile([P, KD * D], bf16)

            def emit_group(g):
                """QK projection for 512 positions [g*512, (g+1)*512)."""
                xh_sb = xpool.tile([P, KD * 512], f16, tag="xh")
                xl_sb = xpool.tile([P, KD * 512], f16, tag="xl")
                nc.sync.dma_start(xh_sb[:], xqh[g, :, :])
                nc.sync.dma_start(xl_sb[:], xql[g, :, :])
                # q triplicated at psum rows 0:6 (col group 0), k at rows
                # 32:38 (col group 1) -- the two matmuls run concurrently.
                ps = mmpsum.tile([P, 512], f32, space="PSUM", tag="mmps")
                terms_q = ((wqh_sb, xh_sb), (wqh_sb, xl_sb), (wql_sb, xh_sb))
                terms_k = ((wkh_sb, xh_sb), (wkh_sb, xl_sb), (wkl_sb, xh_sb))
                n = len(terms_q) * KD
                i = 0
                for (w, xs) in terms_q:
                    for k in range(KD):
                        nc.tensor.matmul(
                            ps[0:6, :],
                            lhsT=w[:, k * 6:(k + 1) * 6],
                            rhs=xs[:, k * 512:(k + 1) * 512],
                            start=(i == 0), stop=(i == n - 1),
                            tile_position=(0, 0),
                        )
                        i += 1
                i = 0
                for (w, xs) in terms_k:
                    for k in range(KD):
                        nc.tensor.matmul(
                            ps[32:38, :],
                            lhsT=w[:, k * 6:(k + 1) * 6],
                            rhs=xs[:, k * 512:(k + 1) * 512],
                            start=(i == 0), stop=(i == n - 1),
                            tile_position=(0, 32),
                        )
                        i += 1
                c0, c1 = g * 512, (g + 1) * 512
                # q: hi-cast rows 0:6 then lo overwrites rows 0:2 (base 0)
                nc.scalar.copy(qs6[0:6, c0:c1], ps[0:6, :])
                nc.vector.tensor_tensor(
                    out=qs6[0:2, c0:c1], in0=ps[0:2, :], in1=qs6[0:2, c0:c1],
                    op=mybir.AluOpType.subtract,
                )
                # k: stage at rows 32:38 (base-32 ops legal), lo overwrites
                # rows 32:34 -> kst = [kl kh kh]; DMA-remap into ks6 [kh kl kh]
                kst = spool.tile([38, 512], f16, tag="kst")
                nc.scalar.copy(kst[32:38, :], ps[32:38, :])
                nc.vector.tensor_tensor(
                    out=kst[32:34, :], in0=ps[32:34, :], in1=kst[32:34, :],
                    op=mybir.AluOpType.subtract,
                )
                nc.sync.dma_start(ks6[2:6, c0:c1], kst[32:36, :])  # [kl kh]
                nc.sync.dma_start(ks6[0:2, c0:c1], kst[34:36, :])  # kh dup

            def emit_tile(i):
                E = (i + 1) * P
                W = 2 * E
                sc = scpool.tile([P, 2 * MYT * P], f32)
                for (base_src, base_dst, mk) in (
                    (0, 0, dmask_sb),
                    (T // 2, E, tmask_sb),
                ):
                    for c0 in range(0, E, 512):
                        c1 = min(E, c0 + 512)
                        nn = c1 - c0
                        ps = mmpsum.tile([P, 512], f32, space="PSUM",
                                         tag="mmps")
                        nc.tensor.matmul(
                            ps[0:P, :nn],
                            lhsT=qs6[0:6, i * P:(i + 1) * P],
                            rhs=ks6[0:6, base_src + c0:base_src + c1],
                            start=True, stop=True,
                        )
                        if c1 == E:
                            if nn > P:
                                nc.scalar.copy(
                                    sc[:, base_dst + c0:base_dst + c1 - P],
                                    ps[0:P, :nn - P],
                                )
                            nc.vector.tensor_tensor(
                                out=sc[:, base_dst + E - P:base_dst + E],
                                in0=ps[0:P, nn - P:nn],
                                in1=mk[:],
                                op=mybir.AluOpType.add,
                            )
                        else:
                            nc.scalar.copy(
                                sc[:, base_dst + c0:base_dst + c1],
                                ps[0:P, :nn],
                            )

                mx8 = spool.tile([P, 8], f32, tag="mx8")
                ix8 = spool.tile([P, 8], u32, tag="ix8")
                nc.vector.max(out=mx8[:], in_=sc[:, :W])
                nc.vector.max_index(out=ix8[:], in_max=mx8[:],
                                    in_values=sc[:, :W])

                # positions >= E belong to range B: add (2048 - E)
                idxf = spool.tile([P, 1], f32, tag="idxf")
                gef = spool.tile([P, 1], f32, tag="gef")
                idxu = spool.tile([P, 1], u32, tag="idxu")
                nc.vector.tensor_copy(idxf[:], ix8[:, 0:1])
                nc.vector.tensor_scalar(
                    gef[:], idxf[:], float(E), scalar2=None,
                    op0=mybir.AluOpType.is_ge,
                )
                nc.vector.tensor_scalar(
                    gef[:], gef[:], float(T // 2 - E), scalar2=None,
                    op0=mybir.AluOpType.mult,
                )
                nc.vector.tensor_tensor(
                    out=idxf[:], in0=idxf[:], in1=gef[:],
                    op=mybir.AluOpType.add,
                )
                nc.vector.tensor_copy(idxu[:], idxf[:])

                xg = xgpool.tile([P, D], bf16)
                nc.gpsimd.indirect_dma_start(
                    out=xg[:],
                    out_offset=None,
                    in_=xv[:],
                    in_offset=bass.IndirectOffsetOnAxis(ap=idxu[:, 0:1],
                                                        axis=0),
                )

                xgT = xtpool.tile([P, D], bf16, tag="xgt")
                for k in range(KD):
                    tp = tpsum.tile([P, P], bf16, space="PSUM", tag="tp")
                    nc.tensor.transpose(
                        tp[:], xg[:, k * P:(k + 1) * P], ident[:]
                    )
                    nc.vector.tensor_copy(xgT[:, k * P:(k + 1) * P], tp[:])

                vo = vopsum.tile([P, D], f32, space="PSUM")
                for n in range(2):
                    for k in range(KD):
                        nc.tensor.matmul(
                            vo[:, n * 512:(n + 1) * 512],
                            lhsT=xgT[:, k * P:(k + 1) * P],
                            rhs=wv_sb[:, k * D + n * 512:k * D + n * 512 + 512],
                            start=(k == 0),
                            stop=(k == KD - 1),
                        )
                ob = opool.tile([P, D], bf16)
                nc.scalar.copy(ob[:], vo[:])
                nc.sync.dma_start(out[i, :, :], ob[:])

            # interleave: groups (j, j+4) then tiles 4j..4j+3
            for j in range(4):
                emit_group(j)
                emit_group(j + 4)
                if j == 0:
                    # W_V load after first group pair's DMAs are queued
                    for k in range(KD):
                        nc.sync.dma_start(
                            wv_sb[:, k * D:(k + 1) * D],
                            w_vT[k * P:(k + 1) * P, :],
                        )
                for i in range(4 * j, 4 * j + 4):
                    emit_tile(i)

    nc.compile()
    return nc


def get_program():
    if "nc" not in _prog_cache:
        _prog_cache["nc"] = _build_program()
    return _prog_cache["nc"]


def _hilo(a):
    """Exact fp16 hi/lo split: a == hi + lo to ~2^-24."""
    hi = a.astype(np.float16)
    lo = (a - hi.astype(np.float32)).astype(np.float16)
    return hi, lo


def make_core_inputs(x_full, W_Q, W_K, W_V):
    import ml_dtypes

    x_full = np.ascontiguousarray(x_full, dtype=np.float32)
    W_Q = np.asarray(W_Q, np.float32)
    W_K = np.asarray(W_K, np.float32)
    w_vT = np.ascontiguousarray(
        np.asarray(W_V, np.float32).T.astype(ml_dtypes.bfloat16))

    # [D, 6] = W.T triplicated, split hi/lo fp16
    def w6(Wm):
        h, l = _hilo(np.concatenate([Wm.T] * 3, axis=1))
        return np.ascontiguousarray(h), np.ascontiguousarray(l)

    wq6h, wq6l = w6(W_Q)
    wk6h, wk6l = w6(W_K)

    r = np.arange(P)
    dmask = np.where(r[None, :] <= r[:, None], 0.0, NEG).astype(np.float32)

    in_maps = []
    tiles_per_core = []
    for c in range(N_CORES):
        b, h = divmod(c, 2)
        mine = [2 * i + h for i in range(MYT)]
        other = [2 * i + (1 - h) for i in range(MYT)]
        rows = np.concatenate(
            [np.arange(t * P, (t + 1) * P) for t in mine + other]
        )
        xb_perm = np.ascontiguousarray(x_full[b][rows])
        xh, xl = _hilo(xb_perm)
        # transposed group layout [NG, P, KD*512]
        def gl(a):
            return np.ascontiguousarray(
                a.reshape(NG, 512, KD, P).transpose(0, 3, 2, 1)
                .reshape(NG, P, KD * 512))
        tmask = np.full((P, P), NEG if h == 0 else 0.0, dtype=np.float32)
        in_maps.append({
            "xqh": gl(xh), "xql": gl(xl),
            "xv": np.ascontiguousarray(xb_perm.astype(ml_dtypes.bfloat16)),
            "wq6h": wq6h, "wq6l": wq6l, "wk6h": wk6h, "wk6l": wk6l,
            "w_vT": w_vT, "dmask": dmask, "tmask": tmask,
        })
        tiles_per_core.append(mine)
    return in_maps, tiles_per_core


def assemble_output(results, tiles_per_core):
    out_full = np.empty((B, T, D), dtype=np.float32)
    for c in range(N_CORES):
        b = c // 2
        oc = np.asarray(results[c]["out"], dtype=np.float32)
        for i, th in enumerate(tiles_per_core[c]):
            out_full[b, th * P:(th + 1) * P, :] = oc[i]
    return out_full


def kernel(**inputs):
    from concourse.bass_utils import run_bass_kernel_spmd

    x_full = np.asarray(inputs["x"], dtype=np.float32)
    in_maps, tiles_per_core = make_core_inputs(
        x_full, np.asarray(inputs["W_Q"]), np.asarray(inputs["W_K"]),
        np.asarray(inputs["W_V"])
    )
    nc = get_program()
    res = run_bass_kernel_spmd(nc, in_maps, core_ids=list(range(N_CORES)))
    return assemble_output(res.results, tiles_per_core)


# revision 29
# speedup vs baseline: 1.6575x; 1.2347x over previous
"""HardMaxAttention Trainium2 Bass kernel (v2: fp16 hi/lo QK + K=6 scores).

Reference computation (per batch b):
    Q = x @ W_Q.T            (T, 2)
    K = x @ W_K.T            (T, 2)
    scores = Q @ K.T         (T, T), causal-masked (strict upper tri = -inf)
    idx = argmax(scores, -1) (T,)
    out = x[idx] @ W_V.T     (T, D)   [== take_along_axis(V, idx)]

Sharding: 8 cores = 4 batches x 2 t-parity shards (as v1).  Core c gets
batch b=c//2, parity h=c%2; x[b] rows are permuted so own tiles occupy
positions 0..2047, other parity 2048..4095.

Precision scheme (the argmax is intolerant of low-precision scores --
bf16 flips ~90 rows, fp32r ~11; fp32 matmuls cost 4 cycles/row):
  - x and W_Q/W_K are split hi/lo into fp16 on the host (x = xh + xl
    exactly to ~2^-24).  Q^T/K^T accumulate in PSUM fp32 from 3 fp16
    matmul terms (Wh xh + Wh xl + Wl xh); error ~2^-24.
  - The PE computes q rows triplicated (M=6, lhsT cols [W,W,W]) so the
    stacked hi/lo score operands can be extracted with partition-aligned
    casts/subs only: qs = [qh qh ql] (rows 0:6), ks = [kh kl kh] (rows
    32:38 via tile_position col group 1).
  - scores = qh.kh + qh.kl + ql.kh as ONE K=6 fp16 matmul per chunk
    (1 cycle/row); error ~2^-22 -> no argmax flips.
  - V path in bf16; output stored bf16 and upcast on host.
"""

import numpy as np

B, T, D, H = 4, 4096, 1024, 2
P = 128
NT = T // P            # 32 t-tiles per batch
MYT = NT // 2          # 16 t-tiles per core
KD = D // P            # 8 contraction blocks
NG = T // 512          # 8 QK groups (4 own-parity, 4 other-parity)
N_CORES = 8
NEG = -1.0e30

_prog_cache = {}


def _build_program():
    import concourse.bacc as bacc
    import concourse.mybir as mybir
    import concourse.tile as tile
    import concourse.bass as bass
    from concourse.masks import make_identity

    f32 = mybir.dt.float32
    f16 = mybir.dt.float16
    bf16 = mybir.dt.bfloat16
    u32 = mybir.dt.uint32

    nc = bacc.Bacc(None, target_bir_lowering=False)

    # x^T in group layout, fp16 hi/lo: xq*[g, p, k*512+c] = x_perm[g*512+c, k*128+p]
    xqh = nc.dram_tensor("xqh", [NG, P, KD * 512], f16, kind="ExternalInput")
    xql = nc.dram_tensor("xql", [NG, P, KD * 512], f16, kind="ExternalInput")
    # gather + V-projection source
    xv = nc.dram_tensor("xv", [T, D], bf16, kind="ExternalInput")
    # weights [D, 12]: cols = [Wq Wq Wq Wk Wk Wk] (2 cols each), hi/lo fp16
    w12h = nc.dram_tensor("w12h", [D, 12], f16, kind="ExternalInput")
    w12l = nc.dram_tensor("w12l", [D, 12], f16, kind="ExternalInput")
    w_vT = nc.dram_tensor("w_vT", [D, D], bf16, kind="ExternalInput")
    dmask = nc.dram_tensor("dmask", [P, P], f32, kind="ExternalInput")
    tmask = nc.dram_tensor("tmask", [P, P], f32, kind="ExternalInput")
    out = nc.dram_tensor("out", [MYT, P, D], bf16, kind="ExternalOutput")

    # group emission order: own-parity g alternating with other-parity g+4,
    # so tiles 4j..4j+3 unlock after pair (j, j+4).
    def gpair(j):
        return (j, j + 4)

    with tile.TileContext(nc) as tc:
        with (
            tc.tile_pool(name="const", bufs=1) as cpool,
            tc.tile_pool(name="xin", bufs=3) as xpool,
            tc.tile_pool(name="xt", bufs=3) as xtpool,
            tc.tile_pool(name="qk", bufs=1) as qkpool,
            tc.tile_pool(name="sc", bufs=3) as scpool,
            tc.tile_pool(name="small", bufs=4) as spool,
            tc.tile_pool(name="xg", bufs=3) as xgpool,
            tc.tile_pool(name="ob", bufs=3) as opool,
            tc.tile_pool(name="tp_ps", bufs=2, space="PSUM") as tpsum,
            tc.tile_pool(name="mm_ps", bufs=4, space="PSUM") as mmpsum,
            tc.tile_pool(name="vo_ps", bufs=2, space="PSUM") as vopsum,
        ):
            # ---- constants ----
            ident = cpool.tile([P, P], bf16)
            make_identity(nc, ident[:])
            # small/constant DMAs go on the scalar queue so the sync (SP)
            # queue starts the big xq loads immediately
            wh_sb = cpool.tile([P, 12 * KD], f16)
            wl_sb = cpool.tile([P, 12 * KD], f16)
            for k in range(KD):
                nc.scalar.dma_start(wh_sb[:, k * 12:(k + 1) * 12],
                                    w12h[k * P:(k + 1) * P, :])
                nc.scalar.dma_start(wl_sb[:, k * 12:(k + 1) * 12],
                                    w12l[k * P:(k + 1) * P, :])
            dmask_sb = cpool.tile([P, P], f32)
            nc.scalar.dma_start(dmask_sb[:], dmask[:])
            tmask_sb = cpool.tile([P, P], f32)
            nc.scalar.dma_start(tmask_sb[:], tmask[:])

            # stacked hi/lo score operands (both base partition 0), paired
            # rows contract together: qs6 = [ql qh qh], ks6 = [kh kl kh]
            # -> ql.kh + qh.kl + qh.kh
            qs6 = qkpool.tile([6, T], f16, tag="qs6")
            ks6 = qkpool.tile([6, T], f16, tag="ks6")

            wv_sb = cpool.tile([P, KD * D], bf16)

            def emit_group(g):
                """QK projection for 512 positions [g*512, (g+1)*512)."""
                xh_sb = xpool.tile([P, KD * 512], f16, tag="xh")
                xl_sb = xpool.tile([P, KD * 512], f16, tag="xl")
                nc.sync.dma_start(xh_sb[:], xqh[g, :, :])
                nc.sync.dma_start(xl_sb[:], xql[g, :, :])
                # single M=12 matmul per hi/lo term per d-chunk:
                # psum rows 0:12 = [q q q k k k] (pairs), fp32 accumulate
                ps = mmpsum.tile([P, 512], f32, space="PSUM", tag="mmps")
                terms = ((wh_sb, xh_sb), (wh_sb, xl_sb), (wl_sb, xh_sb))
                n = len(terms) * KD
                i = 0
                for (w, xs) in terms:
                    for k in range(KD):
                        nc.tensor.matmul(
                            ps[0:12, :],
                            lhsT=w[:, k * 12:(k + 1) * 12],
                            rhs=xs[:, k * 512:(k + 1) * 512],
                            start=(i == 0), stop=(i == n - 1),
                        )
                        i += 1
                c0, c1 = g * 512, (g + 1) * 512
                # stage hi (fp16 cast) and lo (fp32 - hi) for all 12 rows
                # with base-0 ops, then assemble the stacked operands:
                # qs6 = [ql qh qh], ks6 = [kh kl kh]
                hi12 = spool.tile([12, 512], f16, tag="hi12")
                lo12 = spool.tile([12, 512], f16, tag="lo12")
                nc.scalar.copy(hi12[0:12, :], ps[0:12, :])
                nc.vector.tensor_tensor(
                    out=lo12[0:12, :], in0=ps[0:12, :], in1=hi12[0:12, :],
                    op=mybir.AluOpType.subtract,
                )
                nc.vector.tensor_copy(qs6[0:2, c0:c1], lo12[0:2, :])  # ql
                nc.gpsimd.dma_start(qs6[2:6, c0:c1], hi12[2:6, :])    # qh qh
                nc.gpsimd.dma_start(ks6[0:2, c0:c1], hi12[6:8, :])    # kh
                nc.gpsimd.dma_start(ks6[2:4, c0:c1], lo12[6:8, :])    # kl
                nc.gpsimd.dma_start(ks6[4:6, c0:c1], hi12[8:10, :])   # kh

            def emit_tile(i, cp=[0]):
                E = (i + 1) * P
                W = 2 * E
                sc = scpool.tile([P, 2 * MYT * P], f32)

                def chunk_copy(dst, src):
                    # PSUM->SBUF drain: ACT mostly, DVE for every 4th chunk
                    # (only ACT/DVE can read PSUM)
                    if cp[0] % 4 == 3:
                        nc.vector.tensor_copy(dst, src)
                    else:
                        nc.scalar.copy(dst, src)
                    cp[0] += 1

                for (base_src, base_dst, mk) in (
                    (0, 0, dmask_sb),
                    (T // 2, E, tmask_sb),
                ):
                    for c0 in range(0, E, 512):
                        c1 = min(E, c0 + 512)
                        nn = c1 - c0
                        ps = mmpsum.tile([P, 512], f32, space="PSUM",
                                         tag="mmps")
                        nc.tensor.matmul(
                            ps[0:P, :nn],
                            lhsT=qs6[0:6, i * P:(i + 1) * P],
                            rhs=ks6[0:6, base_src + c0:base_src + c1],
                            start=True, stop=True,
                        )
                        if c1 == E:
                            if nn > P:
                                chunk_copy(
                                    sc[:, base_dst + c0:base_dst + c1 - P],
                                    ps[0:P, :nn - P],
                                )
                            nc.vector.tensor_tensor(
                                out=sc[:, base_dst + E - P:base_dst + E],
                                in0=ps[0:P, nn - P:nn],
                                in1=mk[:],
                                op=mybir.AluOpType.add,
                            )
                        else:
                            chunk_copy(
                                sc[:, base_dst + c0:base_dst + c1],
                                ps[0:P, :nn],
                            )

                mx8 = spool.tile([P, 8], f32, tag="mx8")
                ix8 = spool.tile([P, 8], u32, tag="ix8")
                nc.vector.max(out=mx8[:], in_=sc[:, :W])
                nc.vector.max_index(out=ix8[:], in_max=mx8[:],
                                    in_values=sc[:, :W])

                # positions >= E belong to range B: add (2048 - E)
                idxf = spool.tile([P, 1], f32, tag="idxf")
                gef = spool.tile([P, 1], f32, tag="gef")
                idxu = spool.tile([P, 1], u32, tag="idxu")
                nc.vector.tensor_copy(idxf[:], ix8[:, 0:1])
                nc.vector.tensor_scalar(
                    gef[:], idxf[:], float(E), scalar2=None,
                    op0=mybir.AluOpType.is_ge,
                )
                nc.vector.tensor_scalar(
                    gef[:], gef[:], float(T // 2 - E), scalar2=None,
                    op0=mybir.AluOpType.mult,
                )
                nc.vector.tensor_tensor(
                    out=idxf[:], in0=idxf[:], in1=gef[:],
                    op=mybir.AluOpType.add,
                )
                nc.vector.tensor_copy(idxu[:], idxf[:])

                xg = xgpool.tile([P, D], bf16)
                nc.gpsimd.indirect_dma_start(
                    out=xg[:],
                    out_offset=None,
                    in_=xv[:],
                    in_offset=bass.IndirectOffsetOnAxis(ap=idxu[:, 0:1],
                                                        axis=0),
                )

                xgT = xtpool.tile([P, D], bf16, tag="xgt")
                for k in range(KD):
                    tp = tpsum.tile([P, P], bf16, space="PSUM", tag="tp")
                    nc.tensor.transpose(
                        tp[:], xg[:, k * P:(k + 1) * P], ident[:]
                    )
                    nc.vector.tensor_copy(xgT[:, k * P:(k + 1) * P], tp[:])

                ob = opool.tile([P, D], bf16)
                for n in range(2):
                    vo = vopsum.tile([P, 512], f32, space="PSUM", tag="vo")
                    for k in range(KD):
                        nc.tensor.matmul(
                            vo[:],
                            lhsT=xgT[:, k * P:(k + 1) * P],
                            rhs=wv_sb[:, k * D + n * 512:k * D + n * 512 + 512],
                            start=(k == 0),
                            stop=(k == KD - 1),
                        )
                    nc.scalar.copy(ob[:, n * 512:(n + 1) * 512], vo[:])
                nc.sync.dma_start(out[i, :, :], ob[:])

            # interleave: groups (j, j+4) then tiles 4j..4j+3
            for j in range(4):
                emit_group(j)
                emit_group(j + 4)
                if j == 0:
                    # W_V load after first group pair's DMAs are queued;
                    # scalar queue keeps SP free for xq loads
                    for k in range(KD):
                        nc.scalar.dma_start(
                            wv_sb[:, k * D:(k + 1) * D],
                            w_vT[k * P:(k + 1) * P, :],
                        )
                for i in range(4 * j, 4 * j + 4):
                    emit_tile(i)

    nc.compile()
    return nc


def get_program():
    if "nc" not in _prog_cache:
        _prog_cache["nc"] = _build_program()
    return _prog_cache["nc"]


def _hilo(a):
    """Exact fp16 hi/lo split: a == hi + lo to ~2^-24."""
    hi = a.astype(np.float16)
    lo = (a - hi.astype(np.float32)).astype(np.float16)
    return hi, lo


def make_core_inputs(x_full, W_Q, W_K, W_V):
    import ml_dtypes

    x_full = np.ascontiguousarray(x_full, dtype=np.float32)
    W_Q = np.asarray(W_Q, np.float32)
    W_K = np.asarray(W_K, np.float32)
    w_vT = np.ascontiguousarray(
        np.asarray(W_V, np.float32).T.astype(ml_dtypes.bfloat16))

    # [D, 12] = [Wq.T x3 | Wk.T x3], split hi/lo fp16
    w12 = np.concatenate([W_Q.T] * 3 + [W_K.T] * 3, axis=1)  # (D, 12)
    w12h, w12l = _hilo(w12)
    w12h = np.ascontiguousarray(w12h)
    w12l = np.ascontiguousarray(w12l)

    r = np.arange(P)
    dmask = np.where(r[None, :] <= r[:, None], 0.0, NEG).astype(np.float32)

    in_maps = []
    tiles_per_core = []
    for c in range(N_CORES):
        b, h = divmod(c, 2)
        mine = [2 * i + h for i in range(MYT)]
        other = [2 * i + (1 - h) for i in range(MYT)]
        rows = np.concatenate(
            [np.arange(t * P, (t + 1) * P) for t in mine + other]
        )
        xb_perm = np.ascontiguousarray(x_full[b][rows])
        xh, xl = _hilo(xb_perm)
        # transposed group layout [NG, P, KD*512]
        def gl(a):
            return np.ascontiguousarray(
                a.reshape(NG, 512, KD, P).transpose(0, 3, 2, 1)
                .reshape(NG, P, KD * 512))
        tmask = np.full((P, P), NEG if h == 0 else 0.0, dtype=np.float32)
        in_maps.append({
            "xqh": gl(xh), "xql": gl(xl),
            "xv": np.ascontiguousarray(xb_perm.astype(ml_dtypes.bfloat16)),
            "w12h": w12h, "w12l": w12l,
            "w_vT": w_vT, "dmask": dmask, "tmask": tmask,
        })
        tiles_per_core.append(mine)
    return in_maps, tiles_per_core


def assemble_output(results, tiles_per_core):
    out_full = np.empty((B, T, D), dtype=np.float32)
    for c in range(N_CORES):
        b = c // 2
        oc = np.asarray(results[c]["out"], dtype=np.float32)
        for i, th in enumerate(tiles_per_core[c]):
            out_full[b, th * P:(th + 1) * P, :] = oc[i]
    return out_full


def kernel(**inputs):
    from concourse.bass_utils import run_bass_kernel_spmd

    x_full = np.asarray(inputs["x"], dtype=np.float32)
    in_maps, tiles_per_core = make_core_inputs(
        x_full, np.asarray(inputs["W_Q"]), np.asarray(inputs["W_K"]),
        np.asarray(inputs["W_V"])
    )
    nc = get_program()
    res = run_bass_kernel_spmd(nc, in_maps, core_ids=list(range(N_CORES)))
    return assemble_output(res.results, tiles_per_core)


# revision 33
# speedup vs baseline: 1.7350x; 1.0468x over previous
"""HardMaxAttention Trainium2 Bass kernel (v2: fp16 hi/lo QK + K=6 scores).

Reference computation (per batch b):
    Q = x @ W_Q.T            (T, 2)
    K = x @ W_K.T            (T, 2)
    scores = Q @ K.T         (T, T), causal-masked (strict upper tri = -inf)
    idx = argmax(scores, -1) (T,)
    out = x[idx] @ W_V.T     (T, D)   [== take_along_axis(V, idx)]

Sharding: 8 cores = 4 batches x 2 t-parity shards (as v1).  Core c gets
batch b=c//2, parity h=c%2; x[b] rows are permuted so own tiles occupy
positions 0..2047, other parity 2048..4095.

Precision scheme (the argmax is intolerant of low-precision scores --
bf16 flips ~90 rows, fp32r ~11; fp32 matmuls cost 4 cycles/row):
  - x and W_Q/W_K are split hi/lo into fp16 on the host (x = xh + xl
    exactly to ~2^-24).  Q^T/K^T accumulate in PSUM fp32 from 3 fp16
    matmul terms (Wh xh + Wh xl + Wl xh); error ~2^-24.
  - The PE computes q rows triplicated (M=6, lhsT cols [W,W,W]) so the
    stacked hi/lo score operands can be extracted with partition-aligned
    casts/subs only: qs = [qh qh ql] (rows 0:6), ks = [kh kl kh] (rows
    32:38 via tile_position col group 1).
  - scores = qh.kh + qh.kl + ql.kh as ONE K=6 fp16 matmul per chunk
    (1 cycle/row); error ~2^-22 -> no argmax flips.
  - V path in bf16; output stored bf16 and upcast on host.
"""

import numpy as np

B, T, D, H = 4, 4096, 1024, 2
P = 128
NT = T // P            # 32 t-tiles per batch
MYT = NT // 2          # 16 t-tiles per core
KD = D // P            # 8 contraction blocks
NG = T // 512          # 8 QK groups (4 own-parity, 4 other-parity)
N_CORES = 8
NEG = -1.0e30

_prog_cache = {}


def _build_program():
    import concourse.bacc as bacc
    import concourse.mybir as mybir
    import concourse.tile as tile
    import concourse.bass as bass
    from concourse.masks import make_identity

    f32 = mybir.dt.float32
    f16 = mybir.dt.float16
    bf16 = mybir.dt.bfloat16
    u32 = mybir.dt.uint32

    nc = bacc.Bacc(None, target_bir_lowering=False)

    # x^T in group layout, fp16 hi/lo: xq*[g, p, k*512+c] = x_perm[g*512+c, k*128+p]
    xqh = nc.dram_tensor("xqh", [NG, P, KD * 512], f16, kind="ExternalInput")
    xql = nc.dram_tensor("xql", [NG, P, KD * 512], f16, kind="ExternalInput")
    # gather + V-projection source
    xv = nc.dram_tensor("xv", [T, D], bf16, kind="ExternalInput")
    # weights [D, 12]: cols = [Wq Wq Wq Wk Wk Wk] (2 cols each), hi/lo fp16
    w12h = nc.dram_tensor("w12h", [D, 12], f16, kind="ExternalInput")
    w12l = nc.dram_tensor("w12l", [D, 12], f16, kind="ExternalInput")
    w_vT = nc.dram_tensor("w_vT", [D, D], bf16, kind="ExternalInput")
    dmask = nc.dram_tensor("dmask", [P, P], f32, kind="ExternalInput")
    tmask = nc.dram_tensor("tmask", [P, P], f32, kind="ExternalInput")
    out = nc.dram_tensor("out", [MYT, P, D], bf16, kind="ExternalOutput")

    # group emission order: own-parity g alternating with other-parity g+4,
    # so tiles 4j..4j+3 unlock after pair (j, j+4).
    def gpair(j):
        return (j, j + 4)

    with tile.TileContext(nc) as tc:
        with (
            tc.tile_pool(name="const", bufs=1) as cpool,
            tc.tile_pool(name="xin", bufs=3) as xpool,
            tc.tile_pool(name="xt", bufs=3) as xtpool,
            tc.tile_pool(name="qk", bufs=1) as qkpool,
            tc.tile_pool(name="sc", bufs=3) as scpool,
            tc.tile_pool(name="small", bufs=4) as spool,
            tc.tile_pool(name="xg", bufs=3) as xgpool,
            tc.tile_pool(name="ob", bufs=3) as opool,
            tc.tile_pool(name="tp_ps", bufs=2, space="PSUM") as tpsum,
            tc.tile_pool(name="mm_ps", bufs=4, space="PSUM") as mmpsum,
            tc.tile_pool(name="vo_ps", bufs=2, space="PSUM") as vopsum,
        ):
            # ---- constants ----
            ident = cpool.tile([P, P], bf16)
            make_identity(nc, ident[:])
            # small/constant DMAs go on the scalar queue so the sync (SP)
            # queue starts the big xq loads immediately
            wh_sb = cpool.tile([P, 12 * KD], f16)
            wl_sb = cpool.tile([P, 12 * KD], f16)
            for k in range(KD):
                nc.gpsimd.dma_start(wh_sb[:, k * 12:(k + 1) * 12],
                                    w12h[k * P:(k + 1) * P, :])
                nc.gpsimd.dma_start(wl_sb[:, k * 12:(k + 1) * 12],
                                    w12l[k * P:(k + 1) * P, :])
            dmask_sb = cpool.tile([P, P], f32)
            nc.gpsimd.dma_start(dmask_sb[:], dmask[:])
            tmask_sb = cpool.tile([P, P], f32)
            nc.gpsimd.dma_start(tmask_sb[:], tmask[:])

            # stacked hi/lo score operands (both base partition 0), paired
            # rows contract together: qs6 = [ql qh qh], ks6 = [kh kl kh]
            # -> ql.kh + qh.kl + qh.kh
            qs6 = qkpool.tile([6, T], f16, tag="qs6")
            ks6 = qkpool.tile([6, T], f16, tag="ks6")

            wv_sb = cpool.tile([P, KD * D], bf16)

            def emit_group(g):
                """QK projection for 512 positions [g*512, (g+1)*512)."""
                xh_sb = xpool.tile([P, KD * 512], f16, tag="xh")
                xl_sb = xpool.tile([P, KD * 512], f16, tag="xl")
                nc.sync.dma_start(xh_sb[:], xqh[g, :, :])
                nc.sync.dma_start(xl_sb[:], xql[g, :, :])
                # single M=12 matmul per hi/lo term per d-chunk:
                # psum rows 0:12 = [q q q k k k] (pairs), fp32 accumulate
                ps = mmpsum.tile([P, 512], f32, space="PSUM", tag="mmps")
                terms = ((wh_sb, xh_sb), (wh_sb, xl_sb), (wl_sb, xh_sb))
                n = len(terms) * KD
                i = 0
                for (w, xs) in terms:
                    for k in range(KD):
                        nc.tensor.matmul(
                            ps[0:12, :],
                            lhsT=w[:, k * 12:(k + 1) * 12],
                            rhs=xs[:, k * 512:(k + 1) * 512],
                            start=(i == 0), stop=(i == n - 1),
                        )
                        i += 1
                c0, c1 = g * 512, (g + 1) * 512
                # stage hi (fp16 cast) and lo (fp32 - hi) for all 12 rows
                # with base-0 ops, then assemble the stacked operands:
                # qs6 = [ql qh qh], ks6 = [kh kl kh]
                hi12 = spool.tile([12, 512], f16, tag="hi12")
                lo12 = spool.tile([12, 512], f16, tag="lo12")
                nc.scalar.copy(hi12[0:12, :], ps[0:12, :])
                nc.vector.tensor_tensor(
                    out=lo12[0:12, :], in0=ps[0:12, :], in1=hi12[0:12, :],
                    op=mybir.AluOpType.subtract,
                )
                nc.gpsimd.tensor_copy(qs6[0:2, c0:c1], lo12[0:2, :])  # ql
                nc.gpsimd.dma_start(qs6[2:6, c0:c1], hi12[2:6, :])    # qh qh
                nc.gpsimd.dma_start(ks6[0:2, c0:c1], hi12[6:8, :])    # kh
                nc.gpsimd.dma_start(ks6[2:4, c0:c1], lo12[6:8, :])    # kl
                nc.gpsimd.dma_start(ks6[4:6, c0:c1], hi12[8:10, :])   # kh

            xg_tiles = {}

            def emit_scores(i, cp=[0]):
                E = (i + 1) * P
                W = 2 * E
                sc = scpool.tile([P, 2 * MYT * P], f32)

                def chunk_copy(dst, src):
                    # PSUM->SBUF drain: ACT mostly, DVE for every 4th chunk
                    # (only ACT/DVE can read PSUM)
                    if cp[0] % 4 == 3:
                        nc.vector.tensor_copy(dst, src)
                    else:
                        nc.scalar.copy(dst, src)
                    cp[0] += 1

                for (base_src, base_dst, mk) in (
                    (0, 0, dmask_sb),
                    (T // 2, E, tmask_sb),
                ):
                    for c0 in range(0, E, 512):
                        c1 = min(E, c0 + 512)
                        nn = c1 - c0
                        ps = mmpsum.tile([P, 512], f32, space="PSUM",
                                         tag="mmps")
                        nc.tensor.matmul(
                            ps[0:P, :nn],
                            lhsT=qs6[0:6, i * P:(i + 1) * P],
                            rhs=ks6[0:6, base_src + c0:base_src + c1],
                            start=True, stop=True,
                        )
                        if c1 == E:
                            if nn > P:
                                chunk_copy(
                                    sc[:, base_dst + c0:base_dst + c1 - P],
                                    ps[0:P, :nn - P],
                                )
                            nc.vector.tensor_tensor(
                                out=sc[:, base_dst + E - P:base_dst + E],
                                in0=ps[0:P, nn - P:nn],
                                in1=mk[:],
                                op=mybir.AluOpType.add,
                            )
                        else:
                            chunk_copy(
                                sc[:, base_dst + c0:base_dst + c1],
                                ps[0:P, :nn],
                            )

                mx8 = spool.tile([P, 8], f32, tag="mx8")
                ix8 = spool.tile([P, 8], u32, tag="ix8")
                nc.vector.max(out=mx8[:], in_=sc[:, :W])
                nc.vector.max_index(out=ix8[:], in_max=mx8[:],
                                    in_values=sc[:, :W])

                # positions >= E belong to range B: add (2048 - E)
                idxf = spool.tile([P, 1], f32, tag="idxf")
                gef = spool.tile([P, 1], f32, tag="gef")
                idxu = spool.tile([P, 1], u32, tag="idxu")
                nc.vector.tensor_copy(idxf[:], ix8[:, 0:1])
                nc.vector.tensor_scalar(
                    gef[:], idxf[:], float(E), scalar2=None,
                    op0=mybir.AluOpType.is_ge,
                )
                nc.vector.tensor_scalar(
                    gef[:], gef[:], float(T // 2 - E), scalar2=None,
                    op0=mybir.AluOpType.mult,
                )
                nc.vector.tensor_tensor(
                    out=idxf[:], in0=idxf[:], in1=gef[:],
                    op=mybir.AluOpType.add,
                )
                nc.vector.tensor_copy(idxu[:], idxf[:])

                xg = xgpool.tile([P, D], bf16)
                nc.gpsimd.indirect_dma_start(
                    out=xg[:],
                    out_offset=None,
                    in_=xv[:],
                    in_offset=bass.IndirectOffsetOnAxis(ap=idxu[:, 0:1],
                                                        axis=0),
                )
                xg_tiles[i] = xg

            def emit_vproj(i):
                xg = xg_tiles.pop(i)
                # 4 transposes share one PSUM tile -> 1 wide DVE copy per 4
                xgT = xtpool.tile([P, D], bf16, tag="xgt")
                for k4 in range(0, KD, 4):
                    tp = tpsum.tile([P, 512], bf16, space="PSUM", tag="tp")
                    for k in range(4):
                        nc.tensor.transpose(
                            tp[:, k * P:(k + 1) * P],
                            xg[:, (k4 + k) * P:(k4 + k + 1) * P], ident[:]
                        )
                    nc.vector.tensor_copy(
                        xgT[:, k4 * P:(k4 + 4) * P], tp[:])

                ob = opool.tile([P, D], bf16)
                for n in range(2):
                    vo = vopsum.tile([P, 512], f32, space="PSUM", tag="vo")
                    for k in range(KD):
                        nc.tensor.matmul(
                            vo[:],
                            lhsT=xgT[:, k * P:(k + 1) * P],
                            rhs=wv_sb[:, k * D + n * 512:k * D + n * 512 + 512],
                            start=(k == 0),
                            stop=(k == KD - 1),
                        )
                    nc.scalar.copy(ob[:, n * 512:(n + 1) * 512], vo[:])
                nc.sync.dma_start(out[i, :, :], ob[:])

            # software pipeline: scores(i) runs LAG tiles ahead of the
            # transpose+Vproj tail so the PE never waits on a gather
            LAG = 3
            for j in range(4):
                emit_group(j)
                emit_group(j + 4)
                if j == 0:
                    # W_V load after first group pair's DMAs are queued
                    for k in range(KD):
                        nc.gpsimd.dma_start(
                            wv_sb[:, k * D:(k + 1) * D],
                            w_vT[k * P:(k + 1) * P, :],
                        )
                for i in range(4 * j, 4 * j + 4):
                    emit_scores(i)
                    if i - LAG >= 0:
                        emit_vproj(i - LAG)
            for i in range(MYT - LAG, MYT):
                emit_vproj(i)

    nc.compile()
    return nc


def get_program():
    if "nc" not in _prog_cache:
        _prog_cache["nc"] = _build_program()
    return _prog_cache["nc"]


def _hilo(a):
    """Exact fp16 hi/lo split: a == hi + lo to ~2^-24."""
    hi = a.astype(np.float16)
    lo = (a - hi.astype(np.float32)).astype(np.float16)
    return hi, lo


def make_core_inputs(x_full, W_Q, W_K, W_V):
    import ml_dtypes

    x_full = np.ascontiguousarray(x_full, dtype=np.float32)
    W_Q = np.asarray(W_Q, np.float32)
    W_K = np.asarray(W_K, np.float32)
    w_vT = np.ascontiguousarray(
        np.asarray(W_V, np.float32).T.astype(ml_dtypes.bfloat16))

    # [D, 12] = [Wq.T x3 | Wk.T x3], split hi/lo fp16
    w12 = np.concatenate([W_Q.T] * 3 + [W_K.T] * 3, axis=1)  # (D, 12)
    w12h, w12l = _hilo(w12)
    w12h = np.ascontiguousarray(w12h)
    w12l = np.ascontiguousarray(w12l)

    r = np.arange(P)
    dmask = np.where(r[None, :] <= r[:, None], 0.0, NEG).astype(np.float32)

    in_maps = []
    tiles_per_core = []
    for c in range(N_CORES):
        b, h = divmod(c, 2)
        mine = [2 * i + h for i in range(MYT)]
        other = [2 * i + (1 - h) for i in range(MYT)]
        rows = np.concatenate(
            [np.arange(t * P, (t + 1) * P) for t in mine + other]
        )
        xb_perm = np.ascontiguousarray(x_full[b][rows])
        xh, xl = _hilo(xb_perm)
        # transposed group layout [NG, P, KD*512]
        def gl(a):
            return np.ascontiguousarray(
                a.reshape(NG, 512, KD, P).transpose(0, 3, 2, 1)
                .reshape(NG, P, KD * 512))
        tmask = np.full((P, P), NEG if h == 0 else 0.0, dtype=np.float32)
        in_maps.append({
            "xqh": gl(xh), "xql": gl(xl),
            "xv": np.ascontiguousarray(xb_perm.astype(ml_dtypes.bfloat16)),
            "w12h": w12h, "w12l": w12l,
            "w_vT": w_vT, "dmask": dmask, "tmask": tmask,
        })
        tiles_per_core.append(mine)
    return in_maps, tiles_per_core


def assemble_output(results, tiles_per_core):
    out_full = np.empty((B, T, D), dtype=np.float32)
    for c in range(N_CORES):
        b = c // 2
        oc = np.asarray(results[c]["out"], dtype=np.float32)
        for i, th in enumerate(tiles_per_core[c]):
            out_full[b, th * P:(th + 1) * P, :] = oc[i]
    return out_full


def kernel(**inputs):
    from concourse.bass_utils import run_bass_kernel_spmd

    x_full = np.asarray(inputs["x"], dtype=np.float32)
    in_maps, tiles_per_core = make_core_inputs(
        x_full, np.asarray(inputs["W_Q"]), np.asarray(inputs["W_K"]),
        np.asarray(inputs["W_V"])
    )
    nc = get_program()
    res = run_bass_kernel_spmd(nc, in_maps, core_ids=list(range(N_CORES)))
    return assemble_output(res.results, tiles_per_core)


# revision 35
# speedup vs baseline: 1.8062x; 1.0410x over previous
"""HardMaxAttention Trainium2 Bass kernel (v2: fp16 hi/lo QK + K=6 scores).

Reference computation (per batch b):
    Q = x @ W_Q.T            (T, 2)
    K = x @ W_K.T            (T, 2)
    scores = Q @ K.T         (T, T), causal-masked (strict upper tri = -inf)
    idx = argmax(scores, -1) (T,)
    out = x[idx] @ W_V.T     (T, D)   [== take_along_axis(V, idx)]

Sharding: 8 cores = 4 batches x 2 t-parity shards (as v1).  Core c gets
batch b=c//2, parity h=c%2; x[b] rows are permuted so own tiles occupy
positions 0..2047, other parity 2048..4095.

Precision scheme (the argmax is intolerant of low-precision scores --
bf16 flips ~90 rows, fp32r ~11; fp32 matmuls cost 4 cycles/row):
  - x and W_Q/W_K are split hi/lo into fp16 on the host (x = xh + xl
    exactly to ~2^-24).  Q^T/K^T accumulate in PSUM fp32 from 3 fp16
    matmul terms (Wh xh + Wh xl + Wl xh); error ~2^-24.
  - The PE computes q rows triplicated (M=6, lhsT cols [W,W,W]) so the
    stacked hi/lo score operands can be extracted with partition-aligned
    casts/subs only: qs = [qh qh ql] (rows 0:6), ks = [kh kl kh] (rows
    32:38 via tile_position col group 1).
  - scores = qh.kh + qh.kl + ql.kh as ONE K=6 fp16 matmul per chunk
    (1 cycle/row); error ~2^-22 -> no argmax flips.
  - V path in bf16; output stored bf16 and upcast on host.
"""

import numpy as np

B, T, D, H = 4, 4096, 1024, 2
P = 128
NT = T // P            # 32 t-tiles per batch
MYT = NT // 2          # 16 t-tiles per core
KD = D // P            # 8 contraction blocks
NG = T // 512          # 8 QK groups (4 own-parity, 4 other-parity)
N_CORES = 8
NEG = -1.0e30

_prog_cache = {}


def _build_program():
    import concourse.bacc as bacc
    import concourse.mybir as mybir
    import concourse.tile as tile
    import concourse.bass as bass
    from concourse.masks import make_identity

    f32 = mybir.dt.float32
    f16 = mybir.dt.float16
    bf16 = mybir.dt.bfloat16
    u32 = mybir.dt.uint32

    nc = bacc.Bacc(None, target_bir_lowering=False)

    # x^T in group layout, fp16 hi/lo: xq*[g, p, k*512+c] = x_perm[g*512+c, k*128+p]
    xqh = nc.dram_tensor("xqh", [NG, P, KD * 512], f16, kind="ExternalInput")
    xql = nc.dram_tensor("xql", [NG, P, KD * 512], f16, kind="ExternalInput")
    # gather + V-projection source
    xv = nc.dram_tensor("xv", [T, D], bf16, kind="ExternalInput")
    # weights [D, 12]: cols = [Wq Wq Wq Wk Wk Wk] (2 cols each), hi/lo fp16
    w12h = nc.dram_tensor("w12h", [D, 12], f16, kind="ExternalInput")
    w12l = nc.dram_tensor("w12l", [D, 12], f16, kind="ExternalInput")
    w_vT = nc.dram_tensor("w_vT", [D, D], bf16, kind="ExternalInput")
    dmask = nc.dram_tensor("dmask", [P, P], f32, kind="ExternalInput")
    tmask = nc.dram_tensor("tmask", [P, P], f32, kind="ExternalInput")
    out = nc.dram_tensor("out", [MYT, P, D], bf16, kind="ExternalOutput")

    # group emission order: own-parity g alternating with other-parity g+4,
    # so tiles 4j..4j+3 unlock after pair (j, j+4).
    def gpair(j):
        return (j, j + 4)

    with tile.TileContext(nc) as tc:
        with (
            tc.tile_pool(name="const", bufs=1) as cpool,
            tc.tile_pool(name="xin", bufs=3) as xpool,
            tc.tile_pool(name="xt", bufs=3) as xtpool,
            tc.tile_pool(name="qk", bufs=1) as qkpool,
            tc.tile_pool(name="sc", bufs=4) as scpool,
            tc.tile_pool(name="small", bufs=4) as spool,
            tc.tile_pool(name="xg", bufs=3) as xgpool,
            tc.tile_pool(name="ob", bufs=3) as opool,
            tc.tile_pool(name="tp_ps", bufs=2, space="PSUM") as tpsum,
            tc.tile_pool(name="mm_ps", bufs=4, space="PSUM") as mmpsum,
            tc.tile_pool(name="vo_ps", bufs=2, space="PSUM") as vopsum,
        ):
            # ---- constants ----
            ident = cpool.tile([P, P], bf16)
            make_identity(nc, ident[:])
            # small/constant DMAs go on the scalar queue so the sync (SP)
            # queue starts the big xq loads immediately
            wh_sb = cpool.tile([P, 12 * KD], f16)
            wl_sb = cpool.tile([P, 12 * KD], f16)
            for k in range(KD):
                nc.gpsimd.dma_start(wh_sb[:, k * 12:(k + 1) * 12],
                                    w12h[k * P:(k + 1) * P, :])
                nc.gpsimd.dma_start(wl_sb[:, k * 12:(k + 1) * 12],
                                    w12l[k * P:(k + 1) * P, :])
            dmask_sb = cpool.tile([P, P], f32)
            nc.gpsimd.dma_start(dmask_sb[:], dmask[:])
            tmask_sb = cpool.tile([P, P], f32)
            nc.gpsimd.dma_start(tmask_sb[:], tmask[:])

            # stacked hi/lo score operands (both base partition 0), paired
            # rows contract together: qs6 = [ql qh qh], ks6 = [kh kl kh]
            # -> ql.kh + qh.kl + qh.kh
            qs6 = qkpool.tile([6, T], f16, tag="qs6")
            ks6 = qkpool.tile([6, T], f16, tag="ks6")

            wv_sb = cpool.tile([P, KD * D], bf16)

            def emit_group(g):
                """QK projection for 512 positions [g*512, (g+1)*512)."""
                xh_sb = xpool.tile([P, KD * 512], f16, tag="xh")
                xl_sb = xpool.tile([P, KD * 512], f16, tag="xl")
                nc.sync.dma_start(xh_sb[:], xqh[g, :, :])
                nc.scalar.dma_start(xl_sb[:], xql[g, :, :])
                # single M=12 matmul per hi/lo term per d-chunk:
                # psum rows 0:12 = [q q q k k k] (pairs), fp32 accumulate
                ps = mmpsum.tile([P, 512], f32, space="PSUM", tag="mmps")
                terms = ((wh_sb, xh_sb), (wh_sb, xl_sb), (wl_sb, xh_sb))
                n = len(terms) * KD
                i = 0
                for (w, xs) in terms:
                    for k in range(KD):
                        nc.tensor.matmul(
                            ps[0:12, :],
                            lhsT=w[:, k * 12:(k + 1) * 12],
                            rhs=xs[:, k * 512:(k + 1) * 512],
                            start=(i == 0), stop=(i == n - 1),
                        )
                        i += 1
                c0, c1 = g * 512, (g + 1) * 512
                # stage hi (fp16 cast) and lo (fp32 - hi) for all 12 rows
                # with base-0 ops, then assemble the stacked operands:
                # qs6 = [ql qh qh], ks6 = [kh kl kh]
                hi12 = spool.tile([12, 512], f16, tag="hi12")
                lo12 = spool.tile([12, 512], f16, tag="lo12")
                nc.scalar.copy(hi12[0:12, :], ps[0:12, :])
                nc.vector.tensor_tensor(
                    out=lo12[0:12, :], in0=ps[0:12, :], in1=hi12[0:12, :],
                    op=mybir.AluOpType.subtract,
                )
                nc.vector.tensor_copy(qs6[0:2, c0:c1], lo12[0:2, :])   # ql
                nc.sync.dma_start(qs6[2:6, c0:c1], hi12[2:6, :])      # qh qh
                nc.sync.dma_start(ks6[0:2, c0:c1], hi12[6:8, :])      # kh
                nc.sync.dma_start(ks6[2:4, c0:c1], lo12[6:8, :])      # kl
                nc.sync.dma_start(ks6[4:6, c0:c1], hi12[8:10, :])     # kh

            xg_tiles = {}

            def emit_scores(i, cp=[0]):
                E = (i + 1) * P
                W = 2 * E
                sc = scpool.tile([P, 2 * MYT * P], f32)

                def chunk_copy(dst, src):
                    # PSUM->SBUF drain: ACT mostly, DVE for every 4th chunk
                    # (only ACT/DVE can read PSUM)
                    if cp[0] % 4 == 3:
                        nc.vector.tensor_copy(dst, src)
                    else:
                        nc.scalar.copy(dst, src)
                    cp[0] += 1

                for (base_src, base_dst, mk) in (
                    (0, 0, dmask_sb),
                    (T // 2, E, tmask_sb),
                ):
                    for c0 in range(0, E, 512):
                        c1 = min(E, c0 + 512)
                        nn = c1 - c0
                        ps = mmpsum.tile([P, 512], f32, space="PSUM",
                                         tag="mmps")
                        nc.tensor.matmul(
                            ps[0:P, :nn],
                            lhsT=qs6[0:6, i * P:(i + 1) * P],
                            rhs=ks6[0:6, base_src + c0:base_src + c1],
                            start=True, stop=True,
                        )
                        if c1 == E:
                            if nn > P:
                                chunk_copy(
                                    sc[:, base_dst + c0:base_dst + c1 - P],
                                    ps[0:P, :nn - P],
                                )
                            nc.vector.tensor_tensor(
                                out=sc[:, base_dst + E - P:base_dst + E],
                                in0=ps[0:P, nn - P:nn],
                                in1=mk[:],
                                op=mybir.AluOpType.add,
                            )
                        else:
                            chunk_copy(
                                sc[:, base_dst + c0:base_dst + c1],
                                ps[0:P, :nn],
                            )

                mx8 = spool.tile([P, 8], f32, tag="mx8")
                ix8 = spool.tile([P, 8], u32, tag="ix8")
                nc.vector.max(out=mx8[:], in_=sc[:, :W])
                nc.vector.max_index(out=ix8[:], in_max=mx8[:],
                                    in_values=sc[:, :W])

                # positions >= E belong to range B: add (2048 - E)
                idxf = spool.tile([P, 1], f32, tag="idxf")
                gef = spool.tile([P, 1], f32, tag="gef")
                idxu = spool.tile([P, 1], u32, tag="idxu")
                nc.vector.tensor_copy(idxf[:], ix8[:, 0:1])
                nc.vector.tensor_scalar(
                    gef[:], idxf[:], float(E), scalar2=None,
                    op0=mybir.AluOpType.is_ge,
                )
                nc.vector.tensor_scalar(
                    gef[:], gef[:], float(T // 2 - E), scalar2=None,
                    op0=mybir.AluOpType.mult,
                )
                nc.vector.tensor_tensor(
                    out=idxf[:], in0=idxf[:], in1=gef[:],
                    op=mybir.AluOpType.add,
                )
                nc.vector.tensor_copy(idxu[:], idxf[:])

                xg = xgpool.tile([P, D], bf16)
                nc.gpsimd.indirect_dma_start(
                    out=xg[:],
                    out_offset=None,
                    in_=xv[:],
                    in_offset=bass.IndirectOffsetOnAxis(ap=idxu[:, 0:1],
                                                        axis=0),
                )
                xg_tiles[i] = xg

            def emit_vproj(i):
                xg = xg_tiles.pop(i)
                # 4 transposes share one PSUM tile -> 1 wide DVE copy per 4
                xgT = xtpool.tile([P, D], bf16, tag="xgt")
                for k4 in range(0, KD, 4):
                    tp = tpsum.tile([P, 512], bf16, space="PSUM", tag="tp")
                    for k in range(4):
                        nc.tensor.transpose(
                            tp[:, k * P:(k + 1) * P],
                            xg[:, (k4 + k) * P:(k4 + k + 1) * P], ident[:]
                        )
                    nc.vector.tensor_copy(
                        xgT[:, k4 * P:(k4 + 4) * P], tp[:])

                ob = opool.tile([P, D], bf16)
                for n in range(2):
                    vo = vopsum.tile([P, 512], f32, space="PSUM", tag="vo")
                    for k in range(KD):
                        nc.tensor.matmul(
                            vo[:],
                            lhsT=xgT[:, k * P:(k + 1) * P],
                            rhs=wv_sb[:, k * D + n * 512:k * D + n * 512 + 512],
                            start=(k == 0),
                            stop=(k == KD - 1),
                        )
                    nc.scalar.copy(ob[:, n * 512:(n + 1) * 512], vo[:])
                nc.sync.dma_start(out[i, :, :], ob[:])

            # software pipeline: scores(i) runs LAG tiles ahead of the
            # transpose+Vproj tail so the PE never waits on a gather
            LAG = 3
            for j in range(4):
                emit_group(j)
                emit_group(j + 4)
                if j == 0:
                    # W_V load after first group pair's DMAs are queued
                    for k in range(KD):
                        nc.gpsimd.dma_start(
                            wv_sb[:, k * D:(k + 1) * D],
                            w_vT[k * P:(k + 1) * P, :],
                        )
                for i in range(4 * j, 4 * j + 4):
                    emit_scores(i)
                    if i - LAG >= 0:
                        emit_vproj(i - LAG)
            for i in range(MYT - LAG, MYT):
                emit_vproj(i)

    nc.compile()
    return nc


def get_program():
    if "nc" not in _prog_cache:
        _prog_cache["nc"] = _build_program()
    return _prog_cache["nc"]


def _hilo(a):
    """Exact fp16 hi/lo split: a == hi + lo to ~2^-24."""
    hi = a.astype(np.float16)
    lo = (a - hi.astype(np.float32)).astype(np.float16)
    return hi, lo


def make_core_inputs(x_full, W_Q, W_K, W_V):
    import ml_dtypes

    x_full = np.ascontiguousarray(x_full, dtype=np.float32)
    W_Q = np.asarray(W_Q, np.float32)
    W_K = np.asarray(W_K, np.float32)
    w_vT = np.ascontiguousarray(
        np.asarray(W_V, np.float32).T.astype(ml_dtypes.bfloat16))

    # [D, 12] = [Wq.T x3 | Wk.T x3], split hi/lo fp16
    w12 = np.concatenate([W_Q.T] * 3 + [W_K.T] * 3, axis=1)  # (D, 12)
    w12h, w12l = _hilo(w12)
    w12h = np.ascontiguousarray(w12h)
    w12l = np.ascontiguousarray(w12l)

    r = np.arange(P)
    dmask = np.where(r[None, :] <= r[:, None], 0.0, NEG).astype(np.float32)

    in_maps = []
    tiles_per_core = []
    for c in range(N_CORES):
        b, h = divmod(c, 2)
        mine = [2 * i + h for i in range(MYT)]
        other = [2 * i + (1 - h) for i in range(MYT)]
        rows = np.concatenate(
            [np.arange(t * P, (t + 1) * P) for t in mine + other]
        )
        xb_perm = np.ascontiguousarray(x_full[b][rows])
        xh, xl = _hilo(xb_perm)
        # transposed group layout [NG, P, KD*512]
        def gl(a):
            return np.ascontiguousarray(
                a.reshape(NG, 512, KD, P).transpose(0, 3, 2, 1)
                .reshape(NG, P, KD * 512))
        tmask = np.full((P, P), NEG if h == 0 else 0.0, dtype=np.float32)
        in_maps.append({
            "xqh": gl(xh), "xql": gl(xl),
            "xv": np.ascontiguousarray(xb_perm.astype(ml_dtypes.bfloat16)),
            "w12h": w12h, "w12l": w12l,
            "w_vT": w_vT, "dmask": dmask, "tmask": tmask,
        })
        tiles_per_core.append(mine)
    return in_maps, tiles_per_core


def assemble_output(results, tiles_per_core):
    out_full = np.empty((B, T, D), dtype=np.float32)
    for c in range(N_CORES):
        b = c // 2
        oc = np.asarray(results[c]["out"], dtype=np.float32)
        for i, th in enumerate(tiles_per_core[c]):
            out_full[b, th * P:(th + 1) * P, :] = oc[i]
    return out_full


def kernel(**inputs):
    from concourse.bass_utils import run_bass_kernel_spmd

    x_full = np.asarray(inputs["x"], dtype=np.float32)
    in_maps, tiles_per_core = make_core_inputs(
        x_full, np.asarray(inputs["W_Q"]), np.asarray(inputs["W_K"]),
        np.asarray(inputs["W_V"])
    )
    nc = get_program()
    res = run_bass_kernel_spmd(nc, in_maps, core_ids=list(range(N_CORES)))
    return assemble_output(res.results, tiles_per_core)


# revision 36
# speedup vs baseline: 1.8887x; 1.0456x over previous
"""HardMaxAttention Trainium2 Bass kernel (v2: fp16 hi/lo QK + K=6 scores).

Reference computation (per batch b):
    Q = x @ W_Q.T            (T, 2)
    K = x @ W_K.T            (T, 2)
    scores = Q @ K.T         (T, T), causal-masked (strict upper tri = -inf)
    idx = argmax(scores, -1) (T,)
    out = x[idx] @ W_V.T     (T, D)   [== take_along_axis(V, idx)]

Sharding: 8 cores = 4 batches x 2 t-parity shards (as v1).  Core c gets
batch b=c//2, parity h=c%2; x[b] rows are permuted so own tiles occupy
positions 0..2047, other parity 2048..4095.

Precision scheme (the argmax is intolerant of low-precision scores --
bf16 flips ~90 rows, fp32r ~11; fp32 matmuls cost 4 cycles/row):
  - x and W_Q/W_K are split hi/lo into fp16 on the host (x = xh + xl
    exactly to ~2^-24).  Q^T/K^T accumulate in PSUM fp32 from 3 fp16
    matmul terms (Wh xh + Wh xl + Wl xh); error ~2^-24.
  - The PE computes q rows triplicated (M=6, lhsT cols [W,W,W]) so the
    stacked hi/lo score operands can be extracted with partition-aligned
    casts/subs only: qs = [qh qh ql] (rows 0:6), ks = [kh kl kh] (rows
    32:38 via tile_position col group 1).
  - scores = qh.kh + qh.kl + ql.kh as ONE K=6 fp16 matmul per chunk
    (1 cycle/row); error ~2^-22 -> no argmax flips.
  - V path in bf16; output stored bf16 and upcast on host.
"""

import numpy as np

B, T, D, H = 4, 4096, 1024, 2
P = 128
NT = T // P            # 32 t-tiles per batch
MYT = NT // 2          # 16 t-tiles per core
KD = D // P            # 8 contraction blocks
NG = T // 512          # 8 QK groups (4 own-parity, 4 other-parity)
N_CORES = 8
NEG = -1.0e30

_prog_cache = {}


def _build_program():
    import concourse.bacc as bacc
    import concourse.mybir as mybir
    import concourse.tile as tile
    import concourse.bass as bass
    from concourse.masks import make_identity

    f32 = mybir.dt.float32
    f16 = mybir.dt.float16
    bf16 = mybir.dt.bfloat16
    u32 = mybir.dt.uint32

    nc = bacc.Bacc(None, target_bir_lowering=False)

    # x^T in group layout, fp16 hi/lo: xq*[g, p, k*512+c] = x_perm[g*512+c, k*128+p]
    xqh = nc.dram_tensor("xqh", [NG, P, KD * 512], f16, kind="ExternalInput")
    xql = nc.dram_tensor("xql", [NG, P, KD * 512], f16, kind="ExternalInput")
    # gather + V-projection source
    xv = nc.dram_tensor("xv", [T, D], bf16, kind="ExternalInput")
    # weights [D, 12]: cols = [Wq Wq Wq Wk Wk Wk] (2 cols each), hi/lo fp16
    w12h = nc.dram_tensor("w12h", [D, 12], f16, kind="ExternalInput")
    w12l = nc.dram_tensor("w12l", [D, 12], f16, kind="ExternalInput")
    w_vT = nc.dram_tensor("w_vT", [D, D], bf16, kind="ExternalInput")
    dmask = nc.dram_tensor("dmask", [P, P], f32, kind="ExternalInput")
    tmask = nc.dram_tensor("tmask", [P, P], f32, kind="ExternalInput")
    out = nc.dram_tensor("out", [MYT, P, D], bf16, kind="ExternalOutput")

    # group emission order: own-parity g alternating with other-parity g+4,
    # so tiles 4j..4j+3 unlock after pair (j, j+4).
    def gpair(j):
        return (j, j + 4)

    with tile.TileContext(nc) as tc:
        with (
            tc.tile_pool(name="const", bufs=1) as cpool,
            tc.tile_pool(name="xin", bufs=3) as xpool,
            tc.tile_pool(name="xt", bufs=3) as xtpool,
            tc.tile_pool(name="qk", bufs=1) as qkpool,
            tc.tile_pool(name="sc", bufs=4) as scpool,
            tc.tile_pool(name="small", bufs=4) as spool,
            tc.tile_pool(name="xg", bufs=3) as xgpool,
            tc.tile_pool(name="ob", bufs=3) as opool,
            tc.tile_pool(name="tp_ps", bufs=2, space="PSUM") as tpsum,
            tc.tile_pool(name="mm_ps", bufs=4, space="PSUM") as mmpsum,
            tc.tile_pool(name="vo_ps", bufs=2, space="PSUM") as vopsum,
        ):
            # ---- constants ----
            ident = cpool.tile([P, P], bf16)
            make_identity(nc, ident[:])
            # small/constant DMAs go on the scalar queue so the sync (SP)
            # queue starts the big xq loads immediately
            wh_sb = cpool.tile([P, 12 * KD], f16)
            wl_sb = cpool.tile([P, 12 * KD], f16)
            for k in range(KD):
                nc.gpsimd.dma_start(wh_sb[:, k * 12:(k + 1) * 12],
                                    w12h[k * P:(k + 1) * P, :])
                nc.gpsimd.dma_start(wl_sb[:, k * 12:(k + 1) * 12],
                                    w12l[k * P:(k + 1) * P, :])
            dmask_sb = cpool.tile([P, P], f32)
            nc.gpsimd.dma_start(dmask_sb[:], dmask[:])
            tmask_sb = cpool.tile([P, P], f32)
            nc.gpsimd.dma_start(tmask_sb[:], tmask[:])

            # stacked hi/lo score operands (both base partition 0), paired
            # rows contract together: qs6 = [ql qh qh], ks6 = [kh kl kh]
            # -> ql.kh + qh.kl + qh.kh
            qs6 = qkpool.tile([6, T], f16, tag="qs6")
            ks6 = qkpool.tile([6, T], f16, tag="ks6")

            wv_sb = cpool.tile([P, KD * D], bf16)

            # warm the PE (HAM un-throttle) during the initial xq DMA wait:
            # ~5us of dummy matmuls on the already-loaded weight tiles
            wps = mmpsum.tile([P, 512], f32, space="PSUM", tag="mmps")
            for wi in range(24):
                nc.tensor.matmul(
                    wps[0:12, 0:96],
                    lhsT=wh_sb[:, 0:12], rhs=wl_sb[:, 0:96],
                    start=True, stop=True,
                )

            def emit_group(g):
                """QK projection for 512 positions [g*512, (g+1)*512)."""
                xh_sb = xpool.tile([P, KD * 512], f16, tag="xh")
                xl_sb = xpool.tile([P, KD * 512], f16, tag="xl")
                nc.sync.dma_start(xh_sb[:], xqh[g, :, :])
                nc.scalar.dma_start(xl_sb[:], xql[g, :, :])
                # single M=12 matmul per hi/lo term per d-chunk:
                # psum rows 0:12 = [q q q k k k] (pairs), fp32 accumulate
                ps = mmpsum.tile([P, 512], f32, space="PSUM", tag="mmps")
                terms = ((wh_sb, xh_sb), (wh_sb, xl_sb), (wl_sb, xh_sb))
                n = len(terms) * KD
                i = 0
                for (w, xs) in terms:
                    for k in range(KD):
                        nc.tensor.matmul(
                            ps[0:12, :],
                            lhsT=w[:, k * 12:(k + 1) * 12],
                            rhs=xs[:, k * 512:(k + 1) * 512],
                            start=(i == 0), stop=(i == n - 1),
                        )
                        i += 1
                c0, c1 = g * 512, (g + 1) * 512
                # stage hi (fp16 cast) and lo (fp32 - hi) for all 12 rows
                # with base-0 ops, then assemble the stacked operands:
                # qs6 = [ql qh qh], ks6 = [kh kl kh]
                hi12 = spool.tile([12, 512], f16, tag="hi12")
                lo12 = spool.tile([12, 512], f16, tag="lo12")
                nc.scalar.copy(hi12[0:12, :], ps[0:12, :])
                nc.vector.tensor_tensor(
                    out=lo12[0:12, :], in0=ps[0:12, :], in1=hi12[0:12, :],
                    op=mybir.AluOpType.subtract,
                )
                nc.vector.tensor_copy(qs6[0:2, c0:c1], lo12[0:2, :])   # ql
                nc.sync.dma_start(qs6[2:6, c0:c1], hi12[2:6, :])      # qh qh
                nc.scalar.dma_start(ks6[0:2, c0:c1], hi12[6:8, :])    # kh
                nc.sync.dma_start(ks6[2:4, c0:c1], lo12[6:8, :])      # kl
                nc.scalar.dma_start(ks6[4:6, c0:c1], hi12[8:10, :])   # kh

            xg_tiles = {}

            def emit_scores(i, cp=[0]):
                E = (i + 1) * P
                W = 2 * E
                sc = scpool.tile([P, 2 * MYT * P], f32)

                def chunk_copy(dst, src):
                    # PSUM->SBUF drain: ACT mostly, DVE for every 4th chunk
                    # (only ACT/DVE can read PSUM)
                    if cp[0] % 4 == 3:
                        nc.vector.tensor_copy(dst, src)
                    else:
                        nc.scalar.copy(dst, src)
                    cp[0] += 1

                for (base_src, base_dst, mk) in (
                    (0, 0, dmask_sb),
                    (T // 2, E, tmask_sb),
                ):
                    for c0 in range(0, E, 512):
                        c1 = min(E, c0 + 512)
                        nn = c1 - c0
                        ps = mmpsum.tile([P, 512], f32, space="PSUM",
                                         tag="mmps")
                        nc.tensor.matmul(
                            ps[0:P, :nn],
                            lhsT=qs6[0:6, i * P:(i + 1) * P],
                            rhs=ks6[0:6, base_src + c0:base_src + c1],
                            start=True, stop=True,
                        )
                        if c1 == E:
                            if nn > P:
                                chunk_copy(
                                    sc[:, base_dst + c0:base_dst + c1 - P],
                                    ps[0:P, :nn - P],
                                )
                            nc.vector.tensor_tensor(
                                out=sc[:, base_dst + E - P:base_dst + E],
                                in0=ps[0:P, nn - P:nn],
                                in1=mk[:],
                                op=mybir.AluOpType.add,
                            )
                        else:
                            chunk_copy(
                                sc[:, base_dst + c0:base_dst + c1],
                                ps[0:P, :nn],
                            )

                mx8 = spool.tile([P, 8], f32, tag="mx8")
                ix8 = spool.tile([P, 8], u32, tag="ix8")
                nc.vector.max(out=mx8[:], in_=sc[:, :W])
                nc.vector.max_index(out=ix8[:], in_max=mx8[:],
                                    in_values=sc[:, :W])

                # positions >= E belong to range B: add (2048 - E)
                idxf = spool.tile([P, 1], f32, tag="idxf")
                gef = spool.tile([P, 1], f32, tag="gef")
                idxu = spool.tile([P, 1], u32, tag="idxu")
                nc.vector.tensor_copy(idxf[:], ix8[:, 0:1])
                nc.vector.tensor_scalar(
                    gef[:], idxf[:], float(E), scalar2=None,
                    op0=mybir.AluOpType.is_ge,
                )
                nc.vector.tensor_scalar(
                    gef[:], gef[:], float(T // 2 - E), scalar2=None,
                    op0=mybir.AluOpType.mult,
                )
                nc.vector.tensor_tensor(
                    out=idxf[:], in0=idxf[:], in1=gef[:],
                    op=mybir.AluOpType.add,
                )
                nc.vector.tensor_copy(idxu[:], idxf[:])

                xg = xgpool.tile([P, D], bf16)
                nc.gpsimd.indirect_dma_start(
                    out=xg[:],
                    out_offset=None,
                    in_=xv[:],
                    in_offset=bass.IndirectOffsetOnAxis(ap=idxu[:, 0:1],
                                                        axis=0),
                )
                xg_tiles[i] = xg

            def emit_vproj(i):
                xg = xg_tiles.pop(i)
                # 4 transposes share one PSUM tile -> 1 wide DVE copy per 4
                xgT = xtpool.tile([P, D], bf16, tag="xgt")
                for k4 in range(0, KD, 4):
                    tp = tpsum.tile([P, 512], bf16, space="PSUM", tag="tp")
                    for k in range(4):
                        nc.tensor.transpose(
                            tp[:, k * P:(k + 1) * P],
                            xg[:, (k4 + k) * P:(k4 + k + 1) * P], ident[:]
                        )
                    nc.vector.tensor_copy(
                        xgT[:, k4 * P:(k4 + 4) * P], tp[:])

                ob = opool.tile([P, D], bf16)
                for n in range(2):
                    vo = vopsum.tile([P, 512], f32, space="PSUM", tag="vo")
                    for k in range(KD):
                        nc.tensor.matmul(
                            vo[:],
                            lhsT=xgT[:, k * P:(k + 1) * P],
                            rhs=wv_sb[:, k * D + n * 512:k * D + n * 512 + 512],
                            start=(k == 0),
                            stop=(k == KD - 1),
                        )
                    nc.scalar.copy(ob[:, n * 512:(n + 1) * 512], vo[:])
                nc.sync.dma_start(out[i, :, :], ob[:])

            # software pipeline: scores(i) runs LAG tiles ahead of the
            # transpose+Vproj tail so the PE never waits on a gather
            LAG = 3
            for j in range(4):
                emit_group(j)
                emit_group(j + 4)
                if j == 0:
                    # W_V load after first group pair's DMAs are queued
                    for k in range(KD):
                        nc.gpsimd.dma_start(
                            wv_sb[:, k * D:(k + 1) * D],
                            w_vT[k * P:(k + 1) * P, :],
                        )
                for i in range(4 * j, 4 * j + 4):
                    if i - LAG >= 0:
                        emit_vproj(i - LAG)
                    emit_scores(i)
            for i in range(MYT - LAG, MYT):
                emit_vproj(i)

    nc.compile()
    return nc


def get_program():
    if "nc" not in _prog_cache:
        _prog_cache["nc"] = _build_program()
    return _prog_cache["nc"]


def _hilo(a):
    """Exact fp16 hi/lo split: a == hi + lo to ~2^-24."""
    hi = a.astype(np.float16)
    lo = (a - hi.astype(np.float32)).astype(np.float16)
    return hi, lo


def make_core_inputs(x_full, W_Q, W_K, W_V):
    import ml_dtypes

    x_full = np.ascontiguousarray(x_full, dtype=np.float32)
    W_Q = np.asarray(W_Q, np.float32)
    W_K = np.asarray(W_K, np.float32)
    w_vT = np.ascontiguousarray(
        np.asarray(W_V, np.float32).T.astype(ml_dtypes.bfloat16))

    # [D, 12] = [Wq.T x3 | Wk.T x3], split hi/lo fp16
    w12 = np.concatenate([W_Q.T] * 3 + [W_K.T] * 3, axis=1)  # (D, 12)
    w12h, w12l = _hilo(w12)
    w12h = np.ascontiguousarray(w12h)
    w12l = np.ascontiguousarray(w12l)

    r = np.arange(P)
    dmask = np.where(r[None, :] <= r[:, None], 0.0, NEG).astype(np.float32)

    in_maps = []
    tiles_per_core = []
    for c in range(N_CORES):
        b, h = divmod(c, 2)
        mine = [2 * i + h for i in range(MYT)]
        other = [2 * i + (1 - h) for i in range(MYT)]
        rows = np.concatenate(
            [np.arange(t * P, (t + 1) * P) for t in mine + other]
        )
        xb_perm = np.ascontiguousarray(x_full[b][rows])
        xh, xl = _hilo(xb_perm)
        # transposed group layout [NG, P, KD*512]
        def gl(a):
            return np.ascontiguousarray(
                a.reshape(NG, 512, KD, P).transpose(0, 3, 2, 1)
                .reshape(NG, P, KD * 512))
        tmask = np.full((P, P), NEG if h == 0 else 0.0, dtype=np.float32)
        in_maps.append({
            "xqh": gl(xh), "xql": gl(xl),
            "xv": np.ascontiguousarray(xb_perm.astype(ml_dtypes.bfloat16)),
            "w12h": w12h, "w12l": w12l,
            "w_vT": w_vT, "dmask": dmask, "tmask": tmask,
        })
        tiles_per_core.append(mine)
    return in_maps, tiles_per_core


def assemble_output(results, tiles_per_core):
    out_full = np.empty((B, T, D), dtype=np.float32)
    for c in range(N_CORES):
        b = c // 2
        oc = np.asarray(results[c]["out"], dtype=np.float32)
        for i, th in enumerate(tiles_per_core[c]):
            out_full[b, th * P:(th + 1) * P, :] = oc[i]
    return out_full


def kernel(**inputs):
    from concourse.bass_utils import run_bass_kernel_spmd

    x_full = np.asarray(inputs["x"], dtype=np.float32)
    in_maps, tiles_per_core = make_core_inputs(
        x_full, np.asarray(inputs["W_Q"]), np.asarray(inputs["W_K"]),
        np.asarray(inputs["W_V"])
    )
    nc = get_program()
    res = run_bass_kernel_spmd(nc, in_maps, core_ids=list(range(N_CORES)))
    return assemble_output(res.results, tiles_per_core)


# revision 37
# speedup vs baseline: 1.9433x; 1.0289x over previous
"""HardMaxAttention Trainium2 Bass kernel (v2: fp16 hi/lo QK + K=6 scores).

Reference computation (per batch b):
    Q = x @ W_Q.T            (T, 2)
    K = x @ W_K.T            (T, 2)
    scores = Q @ K.T         (T, T), causal-masked (strict upper tri = -inf)
    idx = argmax(scores, -1) (T,)
    out = x[idx] @ W_V.T     (T, D)   [== take_along_axis(V, idx)]

Sharding: 8 cores = 4 batches x 2 t-parity shards (as v1).  Core c gets
batch b=c//2, parity h=c%2; x[b] rows are permuted so own tiles occupy
positions 0..2047, other parity 2048..4095.

Precision scheme (the argmax is intolerant of low-precision scores --
bf16 flips ~90 rows, fp32r ~11; fp32 matmuls cost 4 cycles/row):
  - x and W_Q/W_K are split hi/lo into fp16 on the host (x = xh + xl
    exactly to ~2^-24).  Q^T/K^T accumulate in PSUM fp32 from 3 fp16
    matmul terms (Wh xh + Wh xl + Wl xh); error ~2^-24.
  - The PE computes q rows triplicated (M=6, lhsT cols [W,W,W]) so the
    stacked hi/lo score operands can be extracted with partition-aligned
    casts/subs only: qs = [qh qh ql] (rows 0:6), ks = [kh kl kh] (rows
    32:38 via tile_position col group 1).
  - scores = qh.kh + qh.kl + ql.kh as ONE K=6 fp16 matmul per chunk
    (1 cycle/row); error ~2^-22 -> no argmax flips.
  - V path in bf16; output stored bf16 and upcast on host.
"""

import numpy as np

B, T, D, H = 4, 4096, 1024, 2
P = 128
NT = T // P            # 32 t-tiles per batch
MYT = NT // 2          # 16 t-tiles per core
KD = D // P            # 8 contraction blocks
NG = T // 512          # 8 QK groups (4 own-parity, 4 other-parity)
N_CORES = 8
NEG = -1.0e30

_prog_cache = {}


def _build_program():
    import concourse.bacc as bacc
    import concourse.mybir as mybir
    import concourse.tile as tile
    import concourse.bass as bass
    from concourse.masks import make_identity

    f32 = mybir.dt.float32
    f16 = mybir.dt.float16
    bf16 = mybir.dt.bfloat16
    u32 = mybir.dt.uint32

    nc = bacc.Bacc(None, target_bir_lowering=False)

    # x^T in group layout, fp16 hi/lo: xq*[g, p, k*512+c] = x_perm[g*512+c, k*128+p]
    xqh = nc.dram_tensor("xqh", [NG, P, KD * 512], f16, kind="ExternalInput")
    xql = nc.dram_tensor("xql", [NG, P, KD * 512], f16, kind="ExternalInput")
    # gather + V-projection source
    xv = nc.dram_tensor("xv", [T, D], bf16, kind="ExternalInput")
    # weights [D, 12]: cols = [Wq Wq Wq Wk Wk Wk] (2 cols each), hi/lo fp16
    w12h = nc.dram_tensor("w12h", [D, 12], f16, kind="ExternalInput")
    w12l = nc.dram_tensor("w12l", [D, 12], f16, kind="ExternalInput")
    w_vT = nc.dram_tensor("w_vT", [D, D], bf16, kind="ExternalInput")
    dmask = nc.dram_tensor("dmask", [P, P], f32, kind="ExternalInput")
    tmask = nc.dram_tensor("tmask", [P, P], f32, kind="ExternalInput")
    out = nc.dram_tensor("out", [MYT, P, D], bf16, kind="ExternalOutput")

    # group emission order: own-parity g alternating with other-parity g+4,
    # so tiles 4j..4j+3 unlock after pair (j, j+4).
    def gpair(j):
        return (j, j + 4)

    with tile.TileContext(nc) as tc:
        with (
            tc.tile_pool(name="const", bufs=1) as cpool,
            tc.tile_pool(name="xin", bufs=3) as xpool,
            tc.tile_pool(name="xt", bufs=3) as xtpool,
            tc.tile_pool(name="qk", bufs=1) as qkpool,
            tc.tile_pool(name="sc", bufs=4) as scpool,
            tc.tile_pool(name="small", bufs=4) as spool,
            tc.tile_pool(name="xg", bufs=3) as xgpool,
            tc.tile_pool(name="ob", bufs=3) as opool,
            tc.tile_pool(name="tp_ps", bufs=2, space="PSUM") as tpsum,
            tc.tile_pool(name="mm_ps", bufs=4, space="PSUM") as mmpsum,
            tc.tile_pool(name="vo_ps", bufs=2, space="PSUM") as vopsum,
        ):
            # ---- constants ----
            ident = cpool.tile([P, P], bf16)
            make_identity(nc, ident[:])
            # small/constant DMAs go on the scalar queue so the sync (SP)
            # queue starts the big xq loads immediately
            wh_sb = cpool.tile([P, 12 * KD], f16)
            wl_sb = cpool.tile([P, 12 * KD], f16)
            for k in range(KD):
                nc.gpsimd.dma_start(wh_sb[:, k * 12:(k + 1) * 12],
                                    w12h[k * P:(k + 1) * P, :])
                nc.gpsimd.dma_start(wl_sb[:, k * 12:(k + 1) * 12],
                                    w12l[k * P:(k + 1) * P, :])
            dmask_sb = cpool.tile([P, P], f32)
            nc.gpsimd.dma_start(dmask_sb[:], dmask[:])
            tmask_sb = cpool.tile([P, P], f32)
            nc.gpsimd.dma_start(tmask_sb[:], tmask[:])

            # stacked hi/lo score operands (both base partition 0), paired
            # rows contract together: qs6 = [ql qh qh], ks6 = [kh kl kh]
            # -> ql.kh + qh.kl + qh.kh
            qs6 = qkpool.tile([6, T], f16, tag="qs6")
            ks6 = qkpool.tile([6, T], f16, tag="ks6")

            wv_sb = cpool.tile([P, KD * D], bf16)

            # warm the PE (HAM un-throttle) during the initial xq DMA wait:
            # ~5us of dummy matmuls on the already-loaded weight tiles
            wps = mmpsum.tile([P, 512], f32, space="PSUM", tag="mmps")
            for wi in range(24):
                nc.tensor.matmul(
                    wps[0:12, 0:96],
                    lhsT=wh_sb[:, 0:12], rhs=wl_sb[:, 0:96],
                    start=True, stop=True,
                )

            def emit_group(g):
                """QK projection for 512 positions [g*512, (g+1)*512)."""
                xh_sb = xpool.tile([P, KD * 512], f16, tag="xh")
                xl_sb = xpool.tile([P, KD * 512], f16, tag="xl")
                nc.sync.dma_start(xh_sb[:], xqh[g, :, :])
                nc.scalar.dma_start(xl_sb[:], xql[g, :, :])
                # single M=12 matmul per hi/lo term per d-chunk:
                # psum rows 0:12 = [q q q k k k] (pairs), fp32 accumulate
                ps = mmpsum.tile([P, 512], f32, space="PSUM", tag="mmps")
                terms = ((wh_sb, xh_sb), (wh_sb, xl_sb), (wl_sb, xh_sb))
                n = len(terms) * KD
                i = 0
                for (w, xs) in terms:
                    for k in range(KD):
                        nc.tensor.matmul(
                            ps[0:12, :],
                            lhsT=w[:, k * 12:(k + 1) * 12],
                            rhs=xs[:, k * 512:(k + 1) * 512],
                            start=(i == 0), stop=(i == n - 1),
                        )
                        i += 1
                c0, c1 = g * 512, (g + 1) * 512
                # stage hi (fp16 cast) and lo (fp32 - hi) for all 12 rows
                # with base-0 ops, then assemble the stacked operands:
                # qs6 = [ql qh qh], ks6 = [kh kl kh]
                hi12 = spool.tile([12, 512], f16, tag="hi12")
                lo12 = spool.tile([12, 512], f16, tag="lo12")
                nc.scalar.copy(hi12[0:12, :], ps[0:12, :])
                nc.vector.tensor_tensor(
                    out=lo12[0:12, :], in0=ps[0:12, :], in1=hi12[0:12, :],
                    op=mybir.AluOpType.subtract,
                )
                nc.gpsimd.tensor_copy(qs6[0:2, c0:c1], lo12[0:2, :])  # ql
                nc.sync.dma_start(qs6[2:6, c0:c1], hi12[2:6, :])      # qh qh
                nc.scalar.dma_start(ks6[0:2, c0:c1], hi12[6:8, :])    # kh
                nc.sync.dma_start(ks6[2:4, c0:c1], lo12[6:8, :])      # kl
                nc.scalar.dma_start(ks6[4:6, c0:c1], hi12[8:10, :])   # kh

            xg_tiles = {}

            def emit_scores(i, cp=[0]):
                E = (i + 1) * P
                W = 2 * E
                sc = scpool.tile([P, 2 * MYT * P], f32)

                def chunk_copy(dst, src):
                    # PSUM->SBUF drain: ACT mostly, DVE for every 4th chunk
                    # (only ACT/DVE can read PSUM)
                    if cp[0] % 5 == 4:
                        nc.vector.tensor_copy(dst, src)
                    else:
                        nc.scalar.copy(dst, src)
                    cp[0] += 1

                for (base_src, base_dst, mk) in (
                    (0, 0, dmask_sb),
                    (T // 2, E, tmask_sb),
                ):
                    for c0 in range(0, E, 512):
                        c1 = min(E, c0 + 512)
                        nn = c1 - c0
                        ps = mmpsum.tile([P, 512], f32, space="PSUM",
                                         tag="mmps")
                        nc.tensor.matmul(
                            ps[0:P, :nn],
                            lhsT=qs6[0:6, i * P:(i + 1) * P],
                            rhs=ks6[0:6, base_src + c0:base_src + c1],
                            start=True, stop=True,
                        )
                        if c1 == E:
                            if nn > P:
                                chunk_copy(
                                    sc[:, base_dst + c0:base_dst + c1 - P],
                                    ps[0:P, :nn - P],
                                )
                            nc.vector.tensor_tensor(
                                out=sc[:, base_dst + E - P:base_dst + E],
                                in0=ps[0:P, nn - P:nn],
                                in1=mk[:],
                                op=mybir.AluOpType.add,
                            )
                        else:
                            chunk_copy(
                                sc[:, base_dst + c0:base_dst + c1],
                                ps[0:P, :nn],
                            )

                mx8 = spool.tile([P, 8], f32, tag="mx8")
                ix8 = spool.tile([P, 8], u32, tag="ix8")
                nc.vector.max(out=mx8[:], in_=sc[:, :W])
                nc.vector.max_index(out=ix8[:], in_max=mx8[:],
                                    in_values=sc[:, :W])

                # positions >= E belong to range B: add (2048 - E)
                idxf = spool.tile([P, 1], f32, tag="idxf")
                gef = spool.tile([P, 1], f32, tag="gef")
                idxu = spool.tile([P, 1], u32, tag="idxu")
                nc.gpsimd.tensor_copy(idxf[:], ix8[:, 0:1])
                nc.gpsimd.tensor_scalar(
                    gef[:], idxf[:], float(E), scalar2=None,
                    op0=mybir.AluOpType.is_ge,
                )
                nc.gpsimd.tensor_scalar(
                    gef[:], gef[:], float(T // 2 - E), scalar2=None,
                    op0=mybir.AluOpType.mult,
                )
                nc.gpsimd.tensor_tensor(
                    out=idxf[:], in0=idxf[:], in1=gef[:],
                    op=mybir.AluOpType.add,
                )
                nc.gpsimd.tensor_copy(idxu[:], idxf[:])

                xg = xgpool.tile([P, D], bf16)
                nc.gpsimd.indirect_dma_start(
                    out=xg[:],
                    out_offset=None,
                    in_=xv[:],
                    in_offset=bass.IndirectOffsetOnAxis(ap=idxu[:, 0:1],
                                                        axis=0),
                )
                xg_tiles[i] = xg

            def emit_vproj(i):
                xg = xg_tiles.pop(i)
                # 4 transposes share one PSUM tile -> 1 wide DVE copy per 4
                xgT = xtpool.tile([P, D], bf16, tag="xgt")
                for k4 in range(0, KD, 4):
                    tp = tpsum.tile([P, 512], bf16, space="PSUM", tag="tp")
                    for k in range(4):
                        nc.tensor.transpose(
                            tp[:, k * P:(k + 1) * P],
                            xg[:, (k4 + k) * P:(k4 + k + 1) * P], ident[:]
                        )
                    if k4 == 0:
                        nc.vector.tensor_copy(
                            xgT[:, k4 * P:(k4 + 4) * P], tp[:])
                    else:
                        nc.scalar.copy(xgT[:, k4 * P:(k4 + 4) * P], tp[:])

                ob = opool.tile([P, D], bf16)
                for n in range(2):
                    vo = vopsum.tile([P, 512], f32, space="PSUM", tag="vo")
                    for k in range(KD):
                        nc.tensor.matmul(
                            vo[:],
                            lhsT=xgT[:, k * P:(k + 1) * P],
                            rhs=wv_sb[:, k * D + n * 512:k * D + n * 512 + 512],
                            start=(k == 0),
                            stop=(k == KD - 1),
                        )
                    nc.scalar.copy(ob[:, n * 512:(n + 1) * 512], vo[:])
                nc.sync.dma_start(out[i, :, :], ob[:])

            # software pipeline: scores(i) runs LAG tiles ahead of the
            # transpose+Vproj tail so the PE never waits on a gather
            LAG = 3
            for j in range(4):
                emit_group(j)
                emit_group(j + 4)
                if j == 0:
                    # W_V load after first group pair's DMAs are queued
                    for k in range(KD):
                        nc.gpsimd.dma_start(
                            wv_sb[:, k * D:(k + 1) * D],
                            w_vT[k * P:(k + 1) * P, :],
                        )
                for i in range(4 * j, 4 * j + 4):
                    if i - LAG >= 0:
                        emit_vproj(i - LAG)
                    emit_scores(i)
            for i in range(MYT - LAG, MYT):
                emit_vproj(i)

    nc.compile()
    return nc


def get_program():
    if "nc" not in _prog_cache:
        _prog_cache["nc"] = _build_program()
    return _prog_cache["nc"]


def _hilo(a):
    """Exact fp16 hi/lo split: a == hi + lo to ~2^-24."""
    hi = a.astype(np.float16)
    lo = (a - hi.astype(np.float32)).astype(np.float16)
    return hi, lo


def make_core_inputs(x_full, W_Q, W_K, W_V):
    import ml_dtypes

    x_full = np.ascontiguousarray(x_full, dtype=np.float32)
    W_Q = np.asarray(W_Q, np.float32)
    W_K = np.asarray(W_K, np.float32)
    w_vT = np.ascontiguousarray(
        np.asarray(W_V, np.float32).T.astype(ml_dtypes.bfloat16))

    # [D, 12] = [Wq.T x3 | Wk.T x3], split hi/lo fp16
    w12 = np.concatenate([W_Q.T] * 3 + [W_K.T] * 3, axis=1)  # (D, 12)
    w12h, w12l = _hilo(w12)
    w12h = np.ascontiguousarray(w12h)
    w12l = np.ascontiguousarray(w12l)

    r = np.arange(P)
    dmask = np.where(r[None, :] <= r[:, None], 0.0, NEG).astype(np.float32)

    in_maps = []
    tiles_per_core = []
    for c in range(N_CORES):
        b, h = divmod(c, 2)
        mine = [2 * i + h for i in range(MYT)]
        other = [2 * i + (1 - h) for i in range(MYT)]
        rows = np.concatenate(
            [np.arange(t * P, (t + 1) * P) for t in mine + other]
        )
        xb_perm = np.ascontiguousarray(x_full[b][rows])
        xh, xl = _hilo(xb_perm)
        # transposed group layout [NG, P, KD*512]
        def gl(a):
            return np.ascontiguousarray(
                a.reshape(NG, 512, KD, P).transpose(0, 3, 2, 1)
                .reshape(NG, P, KD * 512))
        tmask = np.full((P, P), NEG if h == 0 else 0.0, dtype=np.float32)
        in_maps.append({
            "xqh": gl(xh), "xql": gl(xl),
            "xv": np.ascontiguousarray(xb_perm.astype(ml_dtypes.bfloat16)),
            "w12h": w12h, "w12l": w12l,
            "w_vT": w_vT, "dmask": dmask, "tmask": tmask,
        })
        tiles_per_core.append(mine)
    return in_maps, tiles_per_core


def assemble_output(results, tiles_per_core):
    out_full = np.empty((B, T, D), dtype=np.float32)
    for c in range(N_CORES):
        b = c // 2
        oc = np.asarray(results[c]["out"], dtype=np.float32)
        for i, th in enumerate(tiles_per_core[c]):
            out_full[b, th * P:(th + 1) * P, :] = oc[i]
    return out_full


def kernel(**inputs):
    from concourse.bass_utils import run_bass_kernel_spmd

    x_full = np.asarray(inputs["x"], dtype=np.float32)
    in_maps, tiles_per_core = make_core_inputs(
        x_full, np.asarray(inputs["W_Q"]), np.asarray(inputs["W_K"]),
        np.asarray(inputs["W_V"])
    )
    nc = get_program()
    res = run_bass_kernel_spmd(nc, in_maps, core_ids=list(range(N_CORES)))
    return assemble_output(res.results, tiles_per_core)
